# revision 9
# baseline (speedup 1.0000x reference)
"""CrossModalAttention fused Bass/Tile kernel for Trainium2 (8 NeuronCores).

Math (per batch b):
    pooled = mean_w x_skel[b]                      # [Cs, Ws]
    k  = Wk @ pooled + bk                          # [Ci, Ws]
    q  = Wq @ x_rgb[b] + bq                        # (never materialized)
    energy = q^T k = x_rgb^T (Wq^T k) + 1 (bq^T k) # [HW, Ws]  <- low-rank trick
    att = softmax(energy, axis=-1)
    v  = Wv @ pooled + bv
    out = gamma * (v @ att^T) + x_rgb
       with v@att^T = (Wv pooled)@att^T + bv (1^T att^T) and rowsum(att)=1,
       so bv is folded in as an extra contraction row of the final matmul.

Sharding: pure data-parallel over batch B=16 -> 2 batches per NeuronCore.
"""

import os
import sys

for _p in ("/opt/trn_rl_repo", "/root/.axon_site/_ro/trn_rl_repo"):
    if os.path.isdir(_p) and _p not in sys.path:
        sys.path.insert(0, _p)

import numpy as np

import concourse.bass as bass  # noqa: F401
import concourse.mybir as mybir
import concourse.tile as tile
from concourse import bacc
from concourse.bass_utils import run_bass_kernel_spmd
from concourse.masks import make_identity

B, Cr, H, W = 16, 1024, 28, 28
Cs, Hs, Ws = 256, 25, 25
Ci = 512
HW = H * W  # 784
SK = Hs * Ws  # 625
N_CORES = 8
BPC = B // N_CORES  # batches per core = 2
MT = 112  # energy m-tile (784 = 7*112)
FP = mybir.dt.float32
AX = mybir.AxisListType
AF = mybir.ActivationFunctionType


def _build():
    nc = bacc.Bacc(None, target_bir_lowering=False)

    x_rgb = nc.dram_tensor("x_rgb", [BPC, Cr, HW], FP, kind="ExternalInput")
    x_skel = nc.dram_tensor("x_skel", [BPC, Cs, SK], FP, kind="ExternalInput")
    Wq_d = nc.dram_tensor("Wq", [Ci, Cr], FP, kind="ExternalInput")
    bq_d = nc.dram_tensor("bq", [Ci], FP, kind="ExternalInput")
    Wk_d = nc.dram_tensor("Wk", [Ci, Cs], FP, kind="ExternalInput")
    bk_d = nc.dram_tensor("bk", [Ci], FP, kind="ExternalInput")
    Wv_d = nc.dram_tensor("Wv", [Cr, Cs], FP, kind="ExternalInput")
    bv_d = nc.dram_tensor("bv", [Cr], FP, kind="ExternalInput")
    gamma_d = nc.dram_tensor("gamma", [1], FP, kind="ExternalInput")
    out_d = nc.dram_tensor("out", [BPC, Cr, HW], FP, kind="ExternalOutput")

    with tile.TileContext(nc) as tc:
        with (
            tc.tile_pool(name="const", bufs=1) as const,
            tc.tile_pool(name="wt", bufs=1) as wt,
            tc.tile_pool(name="xp", bufs=2) as xp,
            tc.tile_pool(name="small", bufs=2) as small,
            tc.tile_pool(name="outp", bufs=3) as outp,
            tc.tile_pool(name="ps1", bufs=4, space="PSUM") as ps1,
            tc.tile_pool(name="psop", bufs=2, space="PSUM") as psop,
        ):
            # ---------------- input DMAs (big ones first) ----------------
            x_sbs, xs_sbs = [], []
            for b in range(BPC):
                x_sb = xp.tile([128, 8, HW], FP, tag="x")
                nc.sync.dma_start(x_sb, x_rgb[b].rearrange("(t p) n -> p t n", p=128))
                xs_sb = xp.tile([128, 2, SK], FP, tag="xs")
                nc.sync.dma_start(xs_sb, x_skel[b].rearrange("(t p) j -> p t j", p=128))
                x_sbs.append(x_sb)
                xs_sbs.append(xs_sb)

            Wq_sb = wt.tile([128, 4, Cr], FP)
            nc.sync.dma_start(Wq_sb, Wq_d.rearrange("(t p) r -> p t r", p=128))
            Wk_sb = wt.tile([128, 4, Cs], FP)
            nc.sync.dma_start(Wk_sb, Wk_d.rearrange("(t p) c -> p t c", p=128))
            Wv_sb = wt.tile([128, 8, Cs], FP)
            nc.sync.dma_start(Wv_sb, Wv_d.rearrange("(t p) c -> p t c", p=128))

            bk_row = const.tile([1, Ci], FP)
            nc.sync.dma_start(bk_row, bk_d[:].rearrange("(o i) -> o i", o=1))
            bq_col = const.tile([128, 4], FP)
            nc.sync.dma_start(bq_col, bq_d[:].rearrange("(t p) -> p t", p=128))
            bv_row = const.tile([1, Cr], FP)
            nc.sync.dma_start(bv_row, bv_d[:].rearrange("(o r) -> o r", o=1))
            gamma_b = const.tile([128, 1], FP)
            nc.sync.dma_start(gamma_b, gamma_d[:].to_broadcast([128, 1]))

            identity = const.tile([128, 128], FP)
            make_identity(nc, identity)
            ones_row = const.tile([1, 128], FP)
            nc.vector.memset(ones_row, 1.0)
            # one-hot row selecting the bias row of vT_aug (col Ws = 1)
            onehot = const.tile([1, Ws + 1], FP)
            nc.vector.memset(onehot, 0.0)
            nc.vector.memset(onehot[0:1, Ws : Ws + 1], 1.0)

            # -------- one-time weight transposes (PE), fold 1/25 pooling --
            # WkT[c, i] = Wk[i, c] / 25 ; WvT[c, r] = Wv[r, c] / 25
            WkT_sb = wt.tile([128, 2, Ci], FP)
            for kt in range(4):
                for ct in range(2):
                    tp_ = ps1.tile([128, 128], FP, tag="ps1")
                    nc.tensor.transpose(
                        tp_, Wk_sb[:, kt, ct * 128 : (ct + 1) * 128], identity
                    )
                    nc.scalar.mul(
                        WkT_sb[:, ct, kt * 128 : (kt + 1) * 128], tp_, 1.0 / Ws
                    )
            WvT_sb = wt.tile([128, 2, Cr], FP)
            for rt in range(8):
                for ct in range(2):
                    tp_ = ps1.tile([128, 128], FP, tag="ps1")
                    nc.tensor.transpose(
                        tp_, Wv_sb[:, rt, ct * 128 : (ct + 1) * 128], identity
                    )
                    nc.scalar.mul(
                        WvT_sb[:, ct, rt * 128 : (rt + 1) * 128], tp_, 1.0 / Ws
                    )

            # ---------------- per-batch pipeline ----------------
            for b in range(BPC):
                x_sb = x_sbs[b]

                # pooled[c, s] = sum_w x_skel[c, s, w]  (the 1/25 lives in WkT/WvT)
                # padded with a zero col so it can act as a 26-row lhsT below
                pooled = small.tile([128, 2, Ws + 1], FP, tag="pooled")
                nc.vector.reduce_sum(
                    pooled[:, :, 0:Ws],
                    xs_sbs[b].rearrange("p t (h w) -> p t h w", w=Ws),
                    axis=AX.X,
                )
                nc.vector.memset(pooled[:, :, Ws : Ws + 1], 0.0)

                # k = WkT.T @ pooled + bk  -> [Ci, Ws] as [128, 4, Ws]
                k_sb = small.tile([128, 4, Ws], FP, tag="k")
                for mt in range(4):
                    kp = ps1.tile([128, Ws], FP, tag="ps1")
                    for ct in range(2):
                        nc.tensor.matmul(
                            kp,
                            WkT_sb[:, ct, mt * 128 : (mt + 1) * 128],
                            pooled[:, ct, 0:Ws],
                            start=(ct == 0),
                            stop=False,
                        )
                    nc.tensor.matmul(
                        kp,
                        bk_row[0:1, mt * 128 : (mt + 1) * 128],
                        ones_row[0:1, 0:Ws],
                        start=False,
                        stop=True,
                    )
                    nc.scalar.copy(k_sb[:, mt, :], kp)

                # beT[0, s] = sum_i bq[i] k[i, s]
                beT = small.tile([1, Ws], FP, tag="beT")
                bp = ps1.tile([1, Ws], FP, tag="ps1")
                for kt in range(4):
                    nc.tensor.matmul(
                        bp,
                        bq_col[:, kt : kt + 1],
                        k_sb[:, kt, :],
                        start=(kt == 0),
                        stop=(kt == 3),
                    )
                nc.scalar.copy(beT, bp)

                # kq = Wq.T @ k -> [Cr, Ws] as [128, 8, Ws]
                kq_sb = small.tile([128, 8, Ws], FP, tag="kq")
                for mt in range(8):
                    qp = ps1.tile([128, Ws], FP, tag="ps1")
                    for kt in range(4):
                        nc.tensor.matmul(
                            qp,
                            Wq_sb[:, kt, mt * 128 : (mt + 1) * 128],
                            k_sb[:, kt, :],
                            start=(kt == 0),
                            stop=(kt == 3),
                        )
                    nc.scalar.copy(kq_sb[:, mt, :], qp)

                # vT_aug: rows 0..24 = (Wv pooled)^T (no bias), row 25 = bv.
                # The bv row is written by a rank-1 one-hot matmul (PE can't
                # start writes at partition 25 from DVE/ACT).
                vT_aug = small.tile([Ws + 1, Cr], FP, tag="vT")
                for nt in range(2):
                    vp = ps1.tile([Ws + 1, 512], FP, tag="ps1")
                    nc.tensor.matmul(
                        vp,
                        onehot[0:1, :],
                        bv_row[0:1, nt * 512 : (nt + 1) * 512],
                        start=True,
                        stop=False,
                    )
                    for ct in range(2):
                        nc.tensor.matmul(
                            vp,
                            pooled[:, ct, :],
                            WvT_sb[:, ct, nt * 512 : (nt + 1) * 512],
                            start=False,
                            stop=(ct == 1),
                        )
                    nc.scalar.copy(vT_aug[:, nt * 512 : (nt + 1) * 512], vp)

                # energy -> softmax -> att^T (augmented with ones row, which
                # rides through the PE transpose as an extra column of att2)
                attT = small.tile([Ws + 1, HW], FP, tag="attT")
                for mt in range(7):
                    ep = ps1.tile([MT, Ws], FP, tag="ps1")
                    for kt in range(8):
                        nc.tensor.matmul(
                            ep,
                            x_sb[:, kt, mt * MT : (mt + 1) * MT],
                            kq_sb[:, kt, :],
                            start=(kt == 0),
                            stop=False,
                        )
                    nc.tensor.matmul(
                        ep,
                        ones_row[0:1, 0:MT],
                        beT[0:1, :],
                        start=False,
                        stop=True,
                    )
                    mx = small.tile([MT, 1], FP, tag="mx")
                    nc.vector.reduce_max(mx, ep, axis=AX.X, negate=True)
                    att = small.tile([MT, Ws], FP, tag="att")
                    ssum = small.tile([MT, 1], FP, tag="ssum")
                    nc.scalar.activation(
                        att, ep, func=AF.Exp, bias=mx, scale=1.0, accum_out=ssum
                    )
                    rs = small.tile([MT, 1], FP, tag="rs")
                    nc.vector.reciprocal(rs, ssum)
                    att2 = small.tile([MT, Ws + 1], FP, tag="att2")
                    nc.vector.tensor_scalar_mul(att2[:, 0:Ws], att, rs)
                    nc.vector.memset(att2[:, Ws : Ws + 1], 1.0)
                    tp_ = ps1.tile([Ws + 1, MT], FP, tag="ps1")
                    nc.tensor.transpose(tp_, att2, identity[0:MT, 0:MT])
                    nc.scalar.copy(attT[:, mt * MT : (mt + 1) * MT], tp_)

                # out = gamma * (vT_aug.T @ attT_aug) + x_rgb
                for rt in range(8):
                    op = psop.tile([128, HW], FP, tag="op")
                    nc.tensor.matmul(
                        op[:, 0:512],
                        vT_aug[:, rt * 128 : (rt + 1) * 128],
                        attT[:, 0:512],
                        start=True,
                        stop=True,
                    )
                    nc.tensor.matmul(
                        op[:, 512:HW],
                        vT_aug[:, rt * 128 : (rt + 1) * 128],
                        attT[:, 512:HW],
                        start=True,
                        stop=True,
                    )
                    t_sb = outp.tile([128, HW], FP, tag="t")
                    nc.scalar.mul(t_sb, op, gamma_b[:, 0:1])
                    o_sb = outp.tile([128, HW], FP, tag="o")
                    nc.vector.tensor_add(o_sb, t_sb, x_sb[:, rt, :])
                    nc.sync.dma_start(
                        out_d[b].rearrange("(t p) n -> p t n", p=128)[:, rt, :], o_sb
                    )

    nc.compile()
    return nc


_NC = None


def _get_nc():
    global _NC
    if _NC is None:
        _NC = _build()
    return _NC


def kernel(x_rgb, x_skel, Wq, bq, Wk, bk, Wv, bv, gamma):
    nc = _get_nc()
    xr = np.ascontiguousarray(x_rgb, dtype=np.float32).reshape(B, Cr, HW)
    xs = np.ascontiguousarray(x_skel, dtype=np.float32).reshape(B, Cs, SK)
    shared = {
        "Wq": np.ascontiguousarray(Wq, dtype=np.float32),
        "bq": np.ascontiguousarray(bq, dtype=np.float32),
        "Wk": np.ascontiguousarray(Wk, dtype=np.float32),
        "bk": np.ascontiguousarray(bk, dtype=np.float32),
        "Wv": np.ascontiguousarray(Wv, dtype=np.float32),
        "bv": np.ascontiguousarray(bv, dtype=np.float32),
        "gamma": np.ascontiguousarray(gamma, dtype=np.float32),
    }
    in_maps = [
        {
            "x_rgb": np.ascontiguousarray(xr[c * BPC : (c + 1) * BPC]),
            "x_skel": np.ascontiguousarray(xs[c * BPC : (c + 1) * BPC]),
            **shared,
        }
        for c in range(N_CORES)
    ]
    res = run_bass_kernel_spmd(nc, in_maps, core_ids=list(range(N_CORES)))
    out = np.concatenate([r["out"] for r in res.results], axis=0)
    return out.reshape(B, Cr, H, W).astype(np.float32)


# revision 13
# speedup vs baseline: 1.1901x; 1.1901x over previous
"""CrossModalAttention fused Bass/Tile kernel for Trainium2 (8 NeuronCores).

Math (per batch b):
    pooled = mean_w x_skel[b]                      # [Cs, Ws]
    k  = Wk @ pooled + bk                          # [Ci, Ws]
    q  = Wq @ x_rgb[b] + bq                        # (never materialized)
    energy = q^T k = x_rgb^T (Wq^T k) + 1 (bq^T k) # [HW, Ws]  <- low-rank trick
    att = softmax(energy, axis=-1)
    v  = Wv @ pooled + bv
    out = gamma * (v @ att^T) + x_rgb

Implementation notes:
  * energy is computed transposed (eT = kq^T-contraction, [Ws, HW]) so the PE
    stationary loads are tiny (25 cols) and the streams are wide (512 cols).
  * softmax runs over the partition axis without max-subtraction (energies
    are O(25), exp stays far below fp32 max):
       E = exp(eT); Eaug = [1|I]^T E  (row 0 = colsums, rows 1..25 = E)
       r = 1/Eaug[0]; attT_aug = Eaug * (ones ⊗ r)
    so attT_aug row 0 == 1, which is exactly the weight the folded bv row
    needs in the output matmul.
  * gamma is folded into Wv^T and bv at setup, so the epilogue is a single
    vector add of the residual per output tile.
  * both batches of a core share the k / bq^T k / Wq^T k projections via a
    52-wide concatenated free axis (26 per batch: col 0 is an aug/pad slot).

Sharding: pure data-parallel over batch B=16 -> 2 batches per NeuronCore.
"""

import os
import sys

for _p in ("/opt/trn_rl_repo", "/root/.axon_site/_ro/trn_rl_repo"):
    if os.path.isdir(_p) and _p not in sys.path:
        sys.path.insert(0, _p)

import numpy as np

import concourse.bass as bass  # noqa: F401
import concourse.mybir as mybir
import concourse.tile as tile
from concourse import bacc
from concourse.bass_utils import run_bass_kernel_spmd
from concourse.masks import make_identity

B, Cr, H, W = 16, 1024, 28, 28
Cs, Hs, Ws = 256, 25, 25
Ci = 512
HW = H * W  # 784
SK = Hs * Ws  # 625
N_CORES = 8
BPC = B // N_CORES  # batches per core = 2
WA = Ws + 1  # 26: per-batch block (col/row 0 = aug slot)
NT = (512, 272)  # free-dim tiling of HW=784, bank-aligned
FP = mybir.dt.float32
AX = mybir.AxisListType
AF = mybir.ActivationFunctionType


def _nt_slices():
    off = 0
    for n in NT:
        yield off, n
        off += n


def _build():
    nc = bacc.Bacc(None, target_bir_lowering=False)

    x_rgb = nc.dram_tensor("x_rgb", [BPC, Cr, HW], FP, kind="ExternalInput")
    x_skel = nc.dram_tensor("x_skel", [BPC, Cs, SK], FP, kind="ExternalInput")
    Wq_d = nc.dram_tensor("Wq", [Ci, Cr], FP, kind="ExternalInput")
    bq_d = nc.dram_tensor("bq", [Ci], FP, kind="ExternalInput")
    Wk_d = nc.dram_tensor("Wk", [Ci, Cs], FP, kind="ExternalInput")
    bk_d = nc.dram_tensor("bk", [Ci], FP, kind="ExternalInput")
    Wv_d = nc.dram_tensor("Wv", [Cr, Cs], FP, kind="ExternalInput")
    bv_d = nc.dram_tensor("bv", [Cr], FP, kind="ExternalInput")
    gamma_d = nc.dram_tensor("gamma", [1], FP, kind="ExternalInput")
    out_d = nc.dram_tensor("out", [BPC, Cr, HW], FP, kind="ExternalOutput")

    with tile.TileContext(nc) as tc:
        with (
            tc.tile_pool(name="const", bufs=1) as const,
            tc.tile_pool(name="wt", bufs=1) as wt,
            tc.tile_pool(name="xp", bufs=2) as xp,
            tc.tile_pool(name="small", bufs=2) as small,
            tc.tile_pool(name="outp", bufs=3) as outp,
            tc.tile_pool(name="ps1", bufs=2, space="PSUM") as ps1,
            tc.tile_pool(name="psA", bufs=1, space="PSUM") as psA,
            tc.tile_pool(name="psop", bufs=2, space="PSUM") as psop,
        ):
            # ---------------- input DMAs (big ones first) ----------------
            # x split in two halves so energy matmuls can start after the
            # first 1.6MB lands.
            x_sbs, xs_sbs = [], []
            for b in range(BPC):
                halves = []
                for h in range(2):
                    x_sb = xp.tile([128, 4, HW], FP, tag=f"x{h}")
                    nc.sync.dma_start(
                        x_sb,
                        x_rgb[b].rearrange("(t p) n -> p t n", p=128)[
                            :, h * 4 : (h + 1) * 4, :
                        ],
                    )
                    halves.append(x_sb)
                x_sbs.append(halves)
                xs_sb = xp.tile([128, 2, SK], FP, tag="xs")
                nc.sync.dma_start(xs_sb, x_skel[b].rearrange("(t p) j -> p t j", p=128))
                xs_sbs.append(xs_sb)

            Wq_sb = wt.tile([128, 4, Cr], FP)
            nc.sync.dma_start(Wq_sb, Wq_d.rearrange("(t p) r -> p t r", p=128))
            Wk_sb = wt.tile([128, 4, Cs], FP)
            nc.sync.dma_start(Wk_sb, Wk_d.rearrange("(t p) c -> p t c", p=128))
            Wv_sb = wt.tile([128, 8, Cs], FP)
            nc.sync.dma_start(Wv_sb, Wv_d.rearrange("(t p) c -> p t c", p=128))

            bk_row = const.tile([1, Ci], FP)
            nc.sync.dma_start(bk_row, bk_d[:].rearrange("(o i) -> o i", o=1))
            bq_col = const.tile([128, 4], FP)
            nc.sync.dma_start(bq_col, bq_d[:].rearrange("(t p) -> p t", p=128))
            bv_row = const.tile([1, Cr], FP)
            nc.sync.dma_start(bv_row, bv_d[:].rearrange("(o r) -> o r", o=1))
            gamma_b = const.tile([128, 1], FP)
            nc.sync.dma_start(gamma_b, gamma_d[:].to_broadcast([128, 1]))

            identity = const.tile([128, 128], FP)
            make_identity(nc, identity)
            ones_row = const.tile([1, 512], FP)
            nc.vector.memset(ones_row, 1.0)
            # one-hot row: col 0 = 1 (selects the bias row of vT_aug)
            onehot = const.tile([1, WA], FP)
            nc.vector.memset(onehot, 0.0)
            nc.vector.memset(onehot[0:1, 0:1], 1.0)
            # aug_sel [25, 26]: col 0 = ones (colsum), cols 1..25 = I_25
            aug_sel = const.tile([Ws, WA], FP)
            nc.gpsimd.memset(aug_sel, 0.0)
            nc.gpsimd.affine_select(
                out=aug_sel[:, 1:WA],
                in_=aug_sel[:, 1:WA],
                compare_op=mybir.AluOpType.not_equal,
                fill=1.0,
                base=0,
                pattern=[[-1, Ws]],
                channel_multiplier=1,
            )
            nc.vector.memset(aug_sel[:, 0:1], 1.0)

            # gamma-scaled bits
            g25 = const.tile([128, 1], FP)
            nc.vector.tensor_scalar_mul(g25, gamma_b, 1.0 / Ws)
            gbv_row = const.tile([1, Cr], FP)
            nc.vector.tensor_scalar_mul(gbv_row, bv_row, gamma_b[0:1, 0:1])

            # -------- one-time weight transposes (PE) --------
            # WkT[c, i] = Wk[i, c] / 25 ; WvTg[c, r] = gamma * Wv[r, c] / 25
            WkT_sb = wt.tile([128, 2, Ci], FP)
            for kt in range(4):
                for ct in range(2):
                    tp_ = ps1.tile([128, 128], FP, tag="ps1")
                    nc.tensor.transpose(
                        tp_, Wk_sb[:, kt, ct * 128 : (ct + 1) * 128], identity
                    )
                    nc.scalar.mul(
                        WkT_sb[:, ct, kt * 128 : (kt + 1) * 128], tp_, 1.0 / Ws
                    )
            WvTg_sb = wt.tile([128, 2, Cr], FP)
            for rt in range(8):
                for ct in range(2):
                    tp_ = ps1.tile([128, 128], FP, tag="ps1")
                    nc.tensor.transpose(
                        tp_, Wv_sb[:, rt, ct * 128 : (ct + 1) * 128], identity
                    )
                    nc.scalar.mul(
                        WvTg_sb[:, ct, rt * 128 : (rt + 1) * 128], tp_, g25[:, 0:1]
                    )

            # ---------------- batch-shared projections ----------------
            # pooled_aug [128, 2, 52]: per-batch 26-block, col 0 zero (aug)
            pooled = small.tile([128, 2, 2 * WA], FP, tag="pooled")
            for b in range(BPC):
                nc.vector.memset(pooled[:, :, b * WA : b * WA + 1], 0.0)
                nc.vector.reduce_sum(
                    pooled[:, :, b * WA + 1 : (b + 1) * WA],
                    xs_sbs[b].rearrange("p t (h w) -> p t h w", w=Ws),
                    axis=AX.X,
                )

            # k = WkT.T @ pooled + bk  (both batches) -> [128, 4, 52]
            k_sb = small.tile([128, 4, 2 * WA], FP, tag="k")
            for mt in range(4):
                kp = ps1.tile([128, 2 * WA], FP, tag="ps1")
                for ct in range(2):
                    nc.tensor.matmul(
                        kp,
                        WkT_sb[:, ct, mt * 128 : (mt + 1) * 128],
                        pooled[:, ct, :],
                        start=(ct == 0),
                        stop=False,
                    )
                nc.tensor.matmul(
                    kp,
                    bk_row[0:1, mt * 128 : (mt + 1) * 128],
                    ones_row[0:1, 0 : 2 * WA],
                    start=False,
                    stop=True,
                )
                nc.scalar.copy(k_sb[:, mt, :], kp)

            # beT[0, .] = bq^T k  (both batches)
            beT = small.tile([1, 2 * WA], FP, tag="beT")
            bp = ps1.tile([1, 2 * WA], FP, tag="ps1")
            for kt in range(4):
                nc.tensor.matmul(
                    bp,
                    bq_col[:, kt : kt + 1],
                    k_sb[:, kt, :],
                    start=(kt == 0),
                    stop=(kt == 3),
                )
            nc.scalar.copy(beT, bp)

            # kq = Wq.T @ k  (both batches) -> [128, 8, 52]
            kq_sb = small.tile([128, 8, 2 * WA], FP, tag="kq")
            for mt in range(8):
                qp = ps1.tile([128, 2 * WA], FP, tag="ps1")
                for kt in range(4):
                    nc.tensor.matmul(
                        qp,
                        Wq_sb[:, kt, mt * 128 : (mt + 1) * 128],
                        k_sb[:, kt, :],
                        start=(kt == 0),
                        stop=(kt == 3),
                    )
                nc.scalar.copy(kq_sb[:, mt, :], qp)

            # ---------------- per-batch attention + output ----------------
            for b in range(BPC):
                sl = slice(b * WA + 1, (b + 1) * WA)  # this batch's 25 cols

                # vT_aug [26, 1024]: row 0 = gamma*bv, rows 1..25 = gamma*v^T
                vT_aug = small.tile([WA, Cr], FP, tag="vT")
                for nof, nn in [(0, 512), (512, 512)]:
                    vp = ps1.tile([WA, 512], FP, tag="ps1")
                    nc.tensor.matmul(
                        vp[:, 0:nn],
                        onehot[0:1, :],
                        gbv_row[0:1, nof : nof + nn],
                        start=True,
                        stop=False,
                    )
                    for ct in range(2):
                        nc.tensor.matmul(
                            vp[:, 0:nn],
                            pooled[:, ct, b * WA : (b + 1) * WA],
                            WvTg_sb[:, ct, nof : nof + nn],
                            start=False,
                            stop=(ct == 1),
                        )
                    nc.scalar.copy(vT_aug[:, nof : nof + nn], vp[:, 0:nn])

                # eT [25, 784] = kq^T-contraction of x + be ⊗ 1
                eT = psA.tile([Ws, HW], FP, tag="psA")
                for nof, nn in _nt_slices():
                    for kt in range(8):
                        nc.tensor.matmul(
                            eT[:, nof : nof + nn],
                            kq_sb[:, kt, sl],
                            x_sbs[b][kt // 4][:, kt % 4, nof : nof + nn],
                            start=(kt == 0),
                            stop=False,
                        )
                    nc.tensor.matmul(
                        eT[:, nof : nof + nn],
                        beT[0:1, sl],
                        ones_row[0:1, 0:nn],
                        start=False,
                        stop=True,
                    )

                # E = exp(eT)  (no max-subtract: |energy| <~ 30, fp32-safe)
                E_sb = small.tile([Ws, HW], FP, tag="E")
                nc.scalar.activation(E_sb, eT, func=AF.Exp, bias=0.0, scale=1.0)

                # Eaug = [1|I]^T E : row 0 = colsums, rows 1..25 = E
                Eaug = psA.tile([WA, HW], FP, tag="psA")
                for nof, nn in _nt_slices():
                    nc.tensor.matmul(
                        Eaug[:, nof : nof + nn],
                        aug_sel,
                        E_sb[:, nof : nof + nn],
                        start=True,
                        stop=True,
                    )
                rrow = small.tile([1, HW], FP, tag="rrow")
                nc.vector.reciprocal(rrow, Eaug[0:1, :])
                Eaug_sb = small.tile([WA, HW], FP, tag="Eaug")
                nc.scalar.copy(Eaug_sb, Eaug)

                # attT_aug = Eaug * (ones ⊗ r): row 0 = 1, rows 1..25 = att^T
                Rb = psA.tile([WA, HW], FP, tag="psA")
                for nof, nn in _nt_slices():
                    nc.tensor.matmul(
                        Rb[:, nof : nof + nn],
                        ones_row[0:1, 0:WA],
                        rrow[0:1, nof : nof + nn],
                        start=True,
                        stop=True,
                    )
                attT = small.tile([WA, HW], FP, tag="attT")
                nc.vector.tensor_mul(attT, Eaug_sb, Rb)

                # out = vT_aug.T @ attT_aug + x_rgb
                for rt in range(8):
                    op = psop.tile([128, HW], FP, tag="op")
                    for nof, nn in _nt_slices():
                        nc.tensor.matmul(
                            op[:, nof : nof + nn],
                            vT_aug[:, rt * 128 : (rt + 1) * 128],
                            attT[:, nof : nof + nn],
                            start=True,
                            stop=True,
                        )
                    o_sb = outp.tile([128, HW], FP, tag="o")
                    nc.vector.tensor_add(
                        o_sb, op, x_sbs[b][rt // 4][:, rt % 4, :]
                    )
                    nc.sync.dma_start(
                        out_d[b].rearrange("(t p) n -> p t n", p=128)[:, rt, :], o_sb
                    )

    nc.compile()
    return nc


_NC = None


def _get_nc():
    global _NC
    if _NC is None:
        _NC = _build()
    return _NC


def kernel(x_rgb, x_skel, Wq, bq, Wk, bk, Wv, bv, gamma):
    nc = _get_nc()
    xr = np.ascontiguousarray(x_rgb, dtype=np.float32).reshape(B, Cr, HW)
    xs = np.ascontiguousarray(x_skel, dtype=np.float32).reshape(B, Cs, SK)
    shared = {
        "Wq": np.ascontiguousarray(Wq, dtype=np.float32),
        "bq": np.ascontiguousarray(bq, dtype=np.float32),
        "Wk": np.ascontiguousarray(Wk, dtype=np.float32),
        "bk": np.ascontiguousarray(bk, dtype=np.float32),
        "Wv": np.ascontiguousarray(Wv, dtype=np.float32),
        "bv": np.ascontiguousarray(bv, dtype=np.float32),
        "gamma": np.ascontiguousarray(gamma, dtype=np.float32),
    }
    in_maps = [
        {
            "x_rgb": np.ascontiguousarray(xr[c * BPC : (c + 1) * BPC]),
            "x_skel": np.ascontiguousarray(xs[c * BPC : (c + 1) * BPC]),
            **shared,
        }
        for c in range(N_CORES)
    ]
    res = run_bass_kernel_spmd(nc, in_maps, core_ids=list(range(N_CORES)))
    out = np.concatenate([r["out"] for r in res.results], axis=0)
    return out.reshape(B, Cr, H, W).astype(np.float32)


# revision 28
# speedup vs baseline: 1.9904x; 1.6724x over previous
"""CrossModalAttention fused Bass/Tile kernel for Trainium2 (8 NeuronCores).

Math (per batch b):
    pooled = mean_w x_skel[b]                      # [Cs, Ws]
    k  = Wk @ pooled + bk                          # [Ci, Ws]
    q  = Wq @ x_rgb[b] + bq                        # (never materialized)
    energy = q^T k = x_rgb^T (Wq^T k) + 1 (bq^T k) # [HW, Ws]  <- low-rank trick
    att = softmax(energy, axis=-1)
    v  = Wv @ pooled + bv
    out = gamma * (v @ att^T) + x_rgb

Implementation notes:
  * energy is computed transposed (eT = kq^T-contraction, [Ws, HW]) so the PE
    stationary loads are tiny (25 cols) and the streams are wide (512 cols).
  * softmax runs over the partition axis without max-subtraction (energies
    are O(25), exp stays far below fp32 max):
       E = exp(eT); Eaug = [1|I]^T E  (row 0 = colsums, rows 1..25 = E)
       r = 1/Eaug[0]; attT_aug = Eaug * (ones ⊗ r)
    so attT_aug row 0 == 1, which is exactly the weight the folded bv row
    needs in the output matmul.
  * gamma is folded into Wv^T and bv at setup, so the epilogue is a single
    vector add of the residual per output tile.
  * both batches of a core share the k / bq^T k / Wq^T k projections via a
    52-wide concatenated free axis (26 per batch: col 0 is an aug/pad slot).

Sharding: pure data-parallel over batch B=16 -> 2 batches per NeuronCore.
"""

import os
import sys

for _p in ("/opt/trn_rl_repo", "/root/.axon_site/_ro/trn_rl_repo"):
    if os.path.isdir(_p) and _p not in sys.path:
        sys.path.insert(0, _p)

import numpy as np

import concourse.bass as bass  # noqa: F401
import concourse.mybir as mybir
import concourse.tile as tile
from concourse import bacc
from concourse.bass_utils import run_bass_kernel_spmd
from concourse.masks import make_identity

B, Cr, H, W = 16, 1024, 28, 28
Cs, Hs, Ws = 256, 25, 25
Ci = 512
HW = H * W  # 784
SK = Hs * Ws  # 625
N_CORES = 8
BPC = B // N_CORES  # batches per core = 2
WA = Ws + 1  # 26: per-batch block (col/row 0 = aug slot)
NT = (512, 272)  # free-dim tiling of HW=784, bank-aligned
FP = mybir.dt.float32
FPR = mybir.dt.float32r
AX = mybir.AxisListType
AF = mybir.ActivationFunctionType


def _r(ap):
    """View an fp32 AP as float32r: single-pass PE matmul (1 cycle/row for
    moving dims >= 256, vs 4 for plain fp32)."""
    return ap.bitcast(FPR)


def _nt_slices():
    off = 0
    for n in NT:
        yield off, n
        off += n


def _build():
    nc = bacc.Bacc(None, target_bir_lowering=False)

    x_rgb = nc.dram_tensor("x_rgb", [BPC, Cr, HW], FP, kind="ExternalInput")
    x_skel = nc.dram_tensor("x_skel", [BPC, Cs, SK], FP, kind="ExternalInput")
    Wq_d = nc.dram_tensor("Wq", [Ci, Cr], FP, kind="ExternalInput")
    bq_d = nc.dram_tensor("bq", [Ci], FP, kind="ExternalInput")
    Wk_d = nc.dram_tensor("Wk", [Ci, Cs], FP, kind="ExternalInput")
    bk_d = nc.dram_tensor("bk", [Ci], FP, kind="ExternalInput")
    Wv_d = nc.dram_tensor("Wv", [Cr, Cs], FP, kind="ExternalInput")
    bv_d = nc.dram_tensor("bv", [Cr], FP, kind="ExternalInput")
    gamma_d = nc.dram_tensor("gamma", [1], FP, kind="ExternalInput")
    out_d = nc.dram_tensor("out", [BPC, Cr, HW], FP, kind="ExternalOutput")

    with tile.TileContext(nc) as tc:
        with (
            # float32r outputs are 4-byte fp32 storage; only the matmul
            # ingest rounds (TF32-style), so this is not a real low-precision
            # accumulation.
            nc.allow_low_precision(reason="float32r tagging of fp32 tiles"),
            tc.tile_pool(name="const", bufs=1) as const,
            tc.tile_pool(name="wt", bufs=1) as wt,
            tc.tile_pool(name="xp", bufs=2) as xp,
            tc.tile_pool(name="small", bufs=2) as small,
            tc.tile_pool(name="outp", bufs=3) as outp,
            tc.tile_pool(name="ps1", bufs=2, space="PSUM") as ps1,
            tc.tile_pool(name="psA", bufs=1, space="PSUM") as psA,
            tc.tile_pool(name="psop", bufs=2, space="PSUM") as psop,
        ):
            # ---------------- input DMAs (big ones first) ----------------
            # x split in two halves so energy matmuls can start after the
            # first 1.6MB lands.
            x_sbs, xs_sbs = [], []
            for b in range(BPC):
                halves = []
                for h in range(2):
                    x_sb = xp.tile([128, 4, HW], FP, tag=f"x{h}")
                    nc.sync.dma_start(
                        _r(x_sb[:]),
                        _r(
                            x_rgb[b].rearrange("(t p) n -> p t n", p=128)[
                                :, h * 4 : (h + 1) * 4, :
                            ]
                        ),
                    )
                    halves.append(x_sb)
                x_sbs.append(halves)
                xs_sb = xp.tile([128, 2, SK], FP, tag="xs")
                nc.sync.dma_start(xs_sb, x_skel[b].rearrange("(t p) j -> p t j", p=128))
                xs_sbs.append(xs_sb)

            Wq_sb = wt.tile([128, 4, Cr], FP)
            nc.sync.dma_start(_r(Wq_sb[:]), _r(Wq_d.rearrange("(t p) r -> p t r", p=128)))
            Wk_sb = wt.tile([128, 4, Cs], FP)
            nc.sync.dma_start(Wk_sb, Wk_d.rearrange("(t p) c -> p t c", p=128))
            Wv_sb = wt.tile([128, 8, Cs], FP)
            nc.sync.dma_start(Wv_sb, Wv_d.rearrange("(t p) c -> p t c", p=128))

            bk_row = const.tile([1, Ci], FP)
            nc.sync.dma_start(_r(bk_row[:]), _r(bk_d[:].rearrange("(o i) -> o i", o=1)))
            bq_col = const.tile([128, 4], FP)
            nc.sync.dma_start(_r(bq_col[:]), _r(bq_d[:].rearrange("(t p) -> p t", p=128)))
            bv_row = const.tile([1, Cr], FP)
            nc.sync.dma_start(bv_row, bv_d[:].rearrange("(o r) -> o r", o=1))
            gamma_b = const.tile([128, 1], FP)
            nc.sync.dma_start(gamma_b, gamma_d[:].to_broadcast([128, 1]))

            identity = const.tile([128, 128], FP)
            make_identity(nc, identity)
            # memset cannot write float32r; build constants in fp32 scratch
            # and ACT-copy into the f32r-tagged tiles.
            ones_tmp = const.tile([1, 512], FP, tag="ones_tmp")
            nc.vector.memset(ones_tmp, 1.0)
            ones_row = const.tile([1, 512], FP)
            nc.scalar.copy(_r(ones_row), ones_tmp)
            # one-hot row: col 0 = 1 (selects the bias row of vT_aug)
            oh_tmp = const.tile([1, WA], FP, tag="oh_tmp")
            nc.vector.memset(oh_tmp, 0.0)
            nc.vector.memset(oh_tmp[0:1, 0:1], 1.0)
            onehot = const.tile([1, WA], FP)
            nc.scalar.copy(_r(onehot), oh_tmp)
            # aug_sel [25, 26]: col 0 = ones (colsum), cols 1..25 = I_25
            as_tmp = const.tile([Ws, WA], FP, tag="as_tmp")
            nc.gpsimd.memset(as_tmp, 0.0)
            nc.gpsimd.affine_select(
                out=as_tmp[:, 1:WA],
                in_=as_tmp[:, 1:WA],
                compare_op=mybir.AluOpType.not_equal,
                fill=1.0,
                base=0,
                pattern=[[-1, Ws]],
                channel_multiplier=1,
            )
            nc.vector.memset(as_tmp[:, 0:1], 1.0)
            aug_sel = const.tile([Ws, WA], FP)
            nc.scalar.copy(_r(aug_sel), as_tmp)
            zcol2 = const.tile([128, 2], FP)
            nc.vector.memset(zcol2, 0.0)

            # gamma-scaled bits
            g25 = const.tile([128, 1], FP)
            nc.vector.tensor_scalar_mul(g25, gamma_b, 1.0 / Ws)
            gbv_row = const.tile([1, Cr], FP)
            nc.vector.tensor_scalar_mul(_r(gbv_row), bv_row, gamma_b[0:1, 0:1])

            # -------- one-time weight transposes (PE) --------
            # WkT[c, i] = Wk[i, c] / 25 ; WvTg[c, r] = gamma * Wv[r, c] / 25
            WkT_sb = wt.tile([128, 2, Ci], FP)
            for kt in range(4):
                for ct in range(2):
                    tp_ = ps1.tile([128, 128], FP, tag="ps1")
                    nc.tensor.transpose(
                        tp_, Wk_sb[:, kt, ct * 128 : (ct + 1) * 128], identity
                    )
                    nc.scalar.mul(
                        _r(WkT_sb[:, ct, kt * 128 : (kt + 1) * 128]), tp_, 1.0 / Ws
                    )
            WvTg_sb = wt.tile([128, 2, Cr], FP)
            for rt in range(8):
                for ct in range(2):
                    tp_ = ps1.tile([128, 128], FP, tag="ps1")
                    nc.tensor.transpose(
                        tp_, Wv_sb[:, rt, ct * 128 : (ct + 1) * 128], identity
                    )
                    nc.scalar.mul(
                        _r(WvTg_sb[:, ct, rt * 128 : (rt + 1) * 128]), tp_, g25[:, 0:1]
                    )

            # ---------------- batch-shared projections ----------------
            # pooled_aug [128, 2, 52]: per-batch 26-block, col 0 zero (aug)
            pooled = small.tile([128, 2, 2 * WA], FP, tag="pooled")
            for b in range(BPC):
                nc.scalar.copy(
                    _r(pooled[:, :, b * WA : b * WA + 1]),
                    zcol2.rearrange("p (t o) -> p t o", o=1),
                )
                nc.vector.reduce_sum(
                    _r(pooled[:, :, b * WA + 1 : (b + 1) * WA]),
                    xs_sbs[b].rearrange("p t (h w) -> p t h w", w=Ws),
                    axis=AX.X,
                )

            # k = WkT.T @ pooled + bk  (both batches) -> [128, 4, 52]
            k_sb = small.tile([128, 4, 2 * WA], FP, tag="k")
            for mt in range(4):
                kp = ps1.tile([128, 2 * WA], FP, tag="ps1")
                for ct in range(2):
                    nc.tensor.matmul(
                        kp,
                        _r(WkT_sb[:, ct, mt * 128 : (mt + 1) * 128]),
                        _r(pooled[:, ct, :]),
                        start=(ct == 0),
                        stop=False,
                    )
                nc.tensor.matmul(
                    kp,
                    _r(bk_row[0:1, mt * 128 : (mt + 1) * 128]),
                    _r(ones_row[0:1, 0 : 2 * WA]),
                    start=False,
                    stop=True,
                )
                nc.scalar.copy(_r(k_sb[:, mt, :]), kp)

            # beT[0, .] = bq^T k  (both batches)
            beT = small.tile([1, 2 * WA], FP, tag="beT")
            bp = ps1.tile([1, 2 * WA], FP, tag="ps1")
            for kt in range(4):
                nc.tensor.matmul(
                    bp,
                    _r(bq_col[:, kt : kt + 1]),
                    _r(k_sb[:, kt, :]),
                    start=(kt == 0),
                    stop=(kt == 3),
                )
            nc.scalar.copy(_r(beT), bp)

            # kq = Wq.T @ k  (both batches) -> [128, 8, 52]
            kq_sb = small.tile([128, 8, 2 * WA], FP, tag="kq")
            for mt in range(8):
                qp = ps1.tile([128, 2 * WA], FP, tag="ps1")
                for kt in range(4):
                    nc.tensor.matmul(
                        qp,
                        _r(Wq_sb[:, kt, mt * 128 : (mt + 1) * 128]),
                        _r(k_sb[:, kt, :]),
                        start=(kt == 0),
                        stop=(kt == 3),
                    )
                nc.scalar.copy(_r(kq_sb[:, mt, :]), qp)

            # ---------------- per-batch attention + output ----------------
            for b in range(BPC):
                sl = slice(b * WA + 1, (b + 1) * WA)  # this batch's 25 cols

                # vT_aug [26, 1024]: row 0 = gamma*bv, rows 1..25 = gamma*v^T
                vT_aug = small.tile([WA, Cr], FP, tag="vT")
                for nof, nn in [(0, 512), (512, 512)]:
                    vp = ps1.tile([WA, 512], FP, tag="ps1")
                    nc.tensor.matmul(
                        vp[:, 0:nn],
                        _r(onehot[0:1, :]),
                        _r(gbv_row[0:1, nof : nof + nn]),
                        start=True,
                        stop=False,
                    )
                    for ct in range(2):
                        nc.tensor.matmul(
                            vp[:, 0:nn],
                            _r(pooled[:, ct, b * WA : (b + 1) * WA]),
                            _r(WvTg_sb[:, ct, nof : nof + nn]),
                            start=False,
                            stop=(ct == 1),
                        )
                    nc.scalar.copy(_r(vT_aug[:, nof : nof + nn]), vp[:, 0:nn])

                # eT [25, 784] = kq^T-contraction of x + be ⊗ 1
                eT = psA.tile([Ws, HW], FP, tag="psA")
                for nof, nn in _nt_slices():
                    for kt in range(8):
                        nc.tensor.matmul(
                            eT[:, nof : nof + nn],
                            _r(kq_sb[:, kt, sl]),
                            _r(x_sbs[b][kt // 4][:, kt % 4, nof : nof + nn]),
                            start=(kt == 0),
                            stop=False,
                        )
                    nc.tensor.matmul(
                        eT[:, nof : nof + nn],
                        _r(beT[0:1, sl]),
                        _r(ones_row[0:1, 0:nn]),
                        start=False,
                        stop=True,
                    )

                # E = exp(eT)  (no max-subtract: |energy| <~ 30, fp32-safe)
                E_sb = small.tile([Ws, HW], FP, tag="E")
                nc.scalar.activation(_r(E_sb), eT, func=AF.Exp, bias=0.0, scale=1.0)

                # Eaug = [1|I]^T E : row 0 = colsums, rows 1..25 = E
                Eaug = psA.tile([WA, HW], FP, tag="psA")
                for nof, nn in _nt_slices():
                    nc.tensor.matmul(
                        Eaug[:, nof : nof + nn],
                        _r(aug_sel),
                        _r(E_sb[:, nof : nof + nn]),
                        start=True,
                        stop=True,
                    )
                Eaug_sb = small.tile([WA, HW], FP, tag="Eaug")
                nc.scalar.copy(_r(Eaug_sb), Eaug)

                # S = ones ⊗ colsums (PE broadcast), then wide approx
                # reciprocal (fp32-only op), then attT = Eaug * (1/S):
                # row 0 = s/s = 1, rows 1..25 = att^T
                Sb = psA.tile([WA, HW], FP, tag="psA")
                for nof, nn in _nt_slices():
                    nc.tensor.matmul(
                        Sb[:, nof : nof + nn],
                        _r(ones_row[0:1, 0:WA]),
                        _r(Eaug_sb[0:1, nof : nof + nn]),
                        start=True,
                        stop=True,
                    )
                Rb = small.tile([WA, HW], FP, tag="Rb")
                nc.vector.reciprocal_approx_fast(Rb, Sb)
                attT = small.tile([WA, HW], FP, tag="attT")
                nc.vector.tensor_mul(_r(attT), Eaug_sb, Rb)

                # out = vT_aug.T @ attT_aug + x_rgb
                for rt in range(8):
                    op = psop.tile([128, HW], FP, tag="op")
                    for nof, nn in _nt_slices():
                        nc.tensor.matmul(
                            op[:, nof : nof + nn],
                            _r(vT_aug[:, rt * 128 : (rt + 1) * 128]),
                            _r(attT[:, nof : nof + nn]),
                            start=True,
                            stop=True,
                        )
                    o_sb = outp.tile([128, HW], FP, tag="o")
                    nc.vector.tensor_add(
                        o_sb, op, x_sbs[b][rt // 4][:, rt % 4, :]
                    )
                    nc.sync.dma_start(
                        out_d[b].rearrange("(t p) n -> p t n", p=128)[:, rt, :], o_sb
                    )

    nc.compile()
    return nc


_NC = None


def _get_nc():
    global _NC
    if _NC is None:
        _NC = _build()
    return _NC


def kernel(x_rgb, x_skel, Wq, bq, Wk, bk, Wv, bv, gamma):
    nc = _get_nc()
    xr = np.ascontiguousarray(x_rgb, dtype=np.float32).reshape(B, Cr, HW)
    xs = np.ascontiguousarray(x_skel, dtype=np.float32).reshape(B, Cs, SK)
    shared = {
        "Wq": np.ascontiguousarray(Wq, dtype=np.float32),
        "bq": np.ascontiguousarray(bq, dtype=np.float32),
        "Wk": np.ascontiguousarray(Wk, dtype=np.float32),
        "bk": np.ascontiguousarray(bk, dtype=np.float32),
        "Wv": np.ascontiguousarray(Wv, dtype=np.float32),
        "bv": np.ascontiguousarray(bv, dtype=np.float32),
        "gamma": np.ascontiguousarray(gamma, dtype=np.float32),
    }
    in_maps = [
        {
            "x_rgb": np.ascontiguousarray(xr[c * BPC : (c + 1) * BPC]),
            "x_skel": np.ascontiguousarray(xs[c * BPC : (c + 1) * BPC]),
            **shared,
        }
        for c in range(N_CORES)
    ]
    res = run_bass_kernel_spmd(nc, in_maps, core_ids=list(range(N_CORES)))
    out = np.concatenate([r["out"] for r in res.results], axis=0)
    return out.reshape(B, Cr, H, W).astype(np.float32)


# revision 29
# speedup vs baseline: 2.4409x; 1.2264x over previous
"""CrossModalAttention fused Bass/Tile kernel for Trainium2 (8 NeuronCores).

Math (per batch b):
    pooled = mean_w x_skel[b]                      # [Cs, Ws]
    k  = Wk @ pooled + bk                          # [Ci, Ws]
    q  = Wq @ x_rgb[b] + bq                        # (never materialized)
    energy = q^T k = x_rgb^T (Wq^T k) + 1 (bq^T k) # [HW, Ws]  <- low-rank trick
    att = softmax(energy, axis=-1)
    v  = Wv @ pooled + bv
    out = gamma * (v @ att^T) + x_rgb

Implementation notes:
  * energy is computed transposed (eT = kq^T-contraction, [Ws, HW]) so the PE
    stationary loads are tiny (25 cols) and the streams are wide (512 cols).
  * softmax runs over the partition axis without max-subtraction (energies
    are O(25), exp stays far below fp32 max):
       E = exp(eT); Eaug = [1|I]^T E  (row 0 = colsums, rows 1..25 = E)
       r = 1/Eaug[0]; attT_aug = Eaug * (ones ⊗ r)
    so attT_aug row 0 == 1, which is exactly the weight the folded bv row
    needs in the output matmul.
  * gamma is folded into Wv^T and bv at setup, so the epilogue is a single
    vector add of the residual per output tile.
  * both batches of a core share the k / bq^T k / Wq^T k projections via a
    52-wide concatenated free axis (26 per batch: col 0 is an aug/pad slot).

Sharding: pure data-parallel over batch B=16 -> 2 batches per NeuronCore.
"""

import os
import sys

for _p in ("/opt/trn_rl_repo", "/root/.axon_site/_ro/trn_rl_repo"):
    if os.path.isdir(_p) and _p not in sys.path:
        sys.path.insert(0, _p)

import numpy as np

import concourse.bass as bass  # noqa: F401
import concourse.mybir as mybir
import concourse.tile as tile
from concourse import bacc
from concourse.bass_utils import run_bass_kernel_spmd
from concourse.masks import make_identity

B, Cr, H, W = 16, 1024, 28, 28
Cs, Hs, Ws = 256, 25, 25
Ci = 512
HW = H * W  # 784
SK = Hs * Ws  # 625
N_CORES = 8
BPC = B // N_CORES  # batches per core = 2
WA = Ws + 1  # 26: per-batch block (col/row 0 = aug slot)
NT = (512, 272)  # free-dim tiling of HW=784, bank-aligned
FP = mybir.dt.float32
FPR = mybir.dt.float32r
AX = mybir.AxisListType
AF = mybir.ActivationFunctionType


def _r(ap):
    """View an fp32 AP as float32r: single-pass PE matmul (1 cycle/row for
    moving dims >= 256, vs 4 for plain fp32)."""
    return ap.bitcast(FPR)


def _nt_slices():
    off = 0
    for n in NT:
        yield off, n
        off += n


def _build():
    nc = bacc.Bacc(None, target_bir_lowering=False)

    x_rgb = nc.dram_tensor("x_rgb", [BPC, Cr, HW], FP, kind="ExternalInput")
    x_skel = nc.dram_tensor("x_skel", [BPC, Cs, SK], FP, kind="ExternalInput")
    Wq_d = nc.dram_tensor("Wq", [Ci, Cr], FP, kind="ExternalInput")
    bq_d = nc.dram_tensor("bq", [Ci], FP, kind="ExternalInput")
    Wk_d = nc.dram_tensor("Wk", [Ci, Cs], FP, kind="ExternalInput")
    bk_d = nc.dram_tensor("bk", [Ci], FP, kind="ExternalInput")
    Wv_d = nc.dram_tensor("Wv", [Cr, Cs], FP, kind="ExternalInput")
    bv_d = nc.dram_tensor("bv", [Cr], FP, kind="ExternalInput")
    gamma_d = nc.dram_tensor("gamma", [1], FP, kind="ExternalInput")
    out_d = nc.dram_tensor("out", [BPC, Cr, HW], FP, kind="ExternalOutput")

    with tile.TileContext(nc) as tc:
        with (
            # float32r outputs are 4-byte fp32 storage; only the matmul
            # ingest rounds (TF32-style), so this is not a real low-precision
            # accumulation.
            nc.allow_low_precision(reason="float32r tagging of fp32 tiles"),
            tc.tile_pool(name="const", bufs=1) as const,
            tc.tile_pool(name="wt", bufs=1) as wt,
            tc.tile_pool(name="xp", bufs=2) as xp,
            tc.tile_pool(name="small", bufs=2) as small,
            tc.tile_pool(name="outp", bufs=3) as outp,
            tc.tile_pool(name="ps1", bufs=2, space="PSUM") as ps1,
            tc.tile_pool(name="psA", bufs=1, space="PSUM") as psA,
            tc.tile_pool(name="psop", bufs=2, space="PSUM") as psop,
        ):
            # ------- input DMAs: weights/consts first (they gate all PE
            # work), then x_skel (gates pooled->k->kq), then the big x_rgb
            # tensors which are only needed as the energy matmuls stream.
            bk_row = const.tile([1, Ci], FP)
            nc.sync.dma_start(_r(bk_row[:]), _r(bk_d[:].rearrange("(o i) -> o i", o=1)))
            bq_col = const.tile([128, 4], FP)
            nc.sync.dma_start(_r(bq_col[:]), _r(bq_d[:].rearrange("(t p) -> p t", p=128)))
            bv_row = const.tile([1, Cr], FP)
            nc.sync.dma_start(bv_row, bv_d[:].rearrange("(o r) -> o r", o=1))
            gamma_b = const.tile([128, 1], FP)
            nc.sync.dma_start(gamma_b, gamma_d[:].to_broadcast([128, 1]))

            Wk_sb = wt.tile([128, 4, Cs], FP)
            nc.sync.dma_start(Wk_sb, Wk_d.rearrange("(t p) c -> p t c", p=128))
            Wv_sb = wt.tile([128, 8, Cs], FP)
            nc.sync.dma_start(Wv_sb, Wv_d.rearrange("(t p) c -> p t c", p=128))
            Wq_sb = wt.tile([128, 4, Cr], FP)
            nc.sync.dma_start(_r(Wq_sb[:]), _r(Wq_d.rearrange("(t p) r -> p t r", p=128)))

            x_sbs, xs_sbs = [], []
            for b in range(BPC):
                xs_sb = xp.tile([128, 2, SK], FP, tag="xs")
                nc.sync.dma_start(xs_sb, x_skel[b].rearrange("(t p) j -> p t j", p=128))
                xs_sbs.append(xs_sb)
            for b in range(BPC):
                halves = []
                for h in range(2):
                    x_sb = xp.tile([128, 4, HW], FP, tag=f"x{h}")
                    nc.sync.dma_start(
                        _r(x_sb[:]),
                        _r(
                            x_rgb[b].rearrange("(t p) n -> p t n", p=128)[
                                :, h * 4 : (h + 1) * 4, :
                            ]
                        ),
                    )
                    halves.append(x_sb)
                x_sbs.append(halves)

            identity = const.tile([128, 128], FP)
            make_identity(nc, identity)
            # memset cannot write float32r; build constants in fp32 scratch
            # and ACT-copy into the f32r-tagged tiles.
            ones_tmp = const.tile([1, 512], FP, tag="ones_tmp")
            nc.vector.memset(ones_tmp, 1.0)
            ones_row = const.tile([1, 512], FP)
            nc.scalar.copy(_r(ones_row), ones_tmp)
            # one-hot row: col 0 = 1 (selects the bias row of vT_aug)
            oh_tmp = const.tile([1, WA], FP, tag="oh_tmp")
            nc.vector.memset(oh_tmp, 0.0)
            nc.vector.memset(oh_tmp[0:1, 0:1], 1.0)
            onehot = const.tile([1, WA], FP)
            nc.scalar.copy(_r(onehot), oh_tmp)
            # aug_sel [25, 26]: col 0 = ones (colsum), cols 1..25 = I_25
            as_tmp = const.tile([Ws, WA], FP, tag="as_tmp")
            nc.gpsimd.memset(as_tmp, 0.0)
            nc.gpsimd.affine_select(
                out=as_tmp[:, 1:WA],
                in_=as_tmp[:, 1:WA],
                compare_op=mybir.AluOpType.not_equal,
                fill=1.0,
                base=0,
                pattern=[[-1, Ws]],
                channel_multiplier=1,
            )
            nc.vector.memset(as_tmp[:, 0:1], 1.0)
            aug_sel = const.tile([Ws, WA], FP)
            nc.scalar.copy(_r(aug_sel), as_tmp)
            zcol2 = const.tile([128, 2], FP)
            nc.vector.memset(zcol2, 0.0)

            # gamma-scaled bits
            g25 = const.tile([128, 1], FP)
            nc.vector.tensor_scalar_mul(g25, gamma_b, 1.0 / Ws)
            gbv_row = const.tile([1, Cr], FP)
            nc.vector.tensor_scalar_mul(_r(gbv_row), bv_row, gamma_b[0:1, 0:1])

            # -------- one-time weight transposes (PE) --------
            # WkT[c, i] = Wk[i, c] / 25 ; WvTg[c, r] = gamma * Wv[r, c] / 25
            WkT_sb = wt.tile([128, 2, Ci], FP)
            for kt in range(4):
                for ct in range(2):
                    tp_ = ps1.tile([128, 128], FP, tag="ps1")
                    nc.tensor.transpose(
                        tp_, Wk_sb[:, kt, ct * 128 : (ct + 1) * 128], identity
                    )
                    nc.scalar.mul(
                        _r(WkT_sb[:, ct, kt * 128 : (kt + 1) * 128]), tp_, 1.0 / Ws
                    )
            WvTg_sb = wt.tile([128, 2, Cr], FP)
            for rt in range(8):
                for ct in range(2):
                    tp_ = ps1.tile([128, 128], FP, tag="ps1")
                    nc.tensor.transpose(
                        tp_, Wv_sb[:, rt, ct * 128 : (ct + 1) * 128], identity
                    )
                    nc.scalar.mul(
                        _r(WvTg_sb[:, ct, rt * 128 : (rt + 1) * 128]), tp_, g25[:, 0:1]
                    )

            # ---------------- batch-shared projections ----------------
            # pooled_aug [128, 2, 52]: per-batch 26-block, col 0 zero (aug)
            pooled = small.tile([128, 2, 2 * WA], FP, tag="pooled")
            for b in range(BPC):
                nc.scalar.copy(
                    _r(pooled[:, :, b * WA : b * WA + 1]),
                    zcol2.rearrange("p (t o) -> p t o", o=1),
                )
                nc.vector.reduce_sum(
                    _r(pooled[:, :, b * WA + 1 : (b + 1) * WA]),
                    xs_sbs[b].rearrange("p t (h w) -> p t h w", w=Ws),
                    axis=AX.X,
                )

            # k = WkT.T @ pooled + bk  (both batches) -> [128, 4, 52]
            k_sb = small.tile([128, 4, 2 * WA], FP, tag="k")
            for mt in range(4):
                kp = ps1.tile([128, 2 * WA], FP, tag="ps1")
                for ct in range(2):
                    nc.tensor.matmul(
                        kp,
                        _r(WkT_sb[:, ct, mt * 128 : (mt + 1) * 128]),
                        _r(pooled[:, ct, :]),
                        start=(ct == 0),
                        stop=False,
                    )
                nc.tensor.matmul(
                    kp,
                    _r(bk_row[0:1, mt * 128 : (mt + 1) * 128]),
                    _r(ones_row[0:1, 0 : 2 * WA]),
                    start=False,
                    stop=True,
                )
                nc.scalar.copy(_r(k_sb[:, mt, :]), kp)

            # beT[0, .] = bq^T k  (both batches)
            beT = small.tile([1, 2 * WA], FP, tag="beT")
            bp = ps1.tile([1, 2 * WA], FP, tag="ps1")
            for kt in range(4):
                nc.tensor.matmul(
                    bp,
                    _r(bq_col[:, kt : kt + 1]),
                    _r(k_sb[:, kt, :]),
                    start=(kt == 0),
                    stop=(kt == 3),
                )
            nc.scalar.copy(_r(beT), bp)

            # kq = Wq.T @ k  (both batches) -> [128, 8, 52]
            kq_sb = small.tile([128, 8, 2 * WA], FP, tag="kq")
            for mt in range(8):
                qp = ps1.tile([128, 2 * WA], FP, tag="ps1")
                for kt in range(4):
                    nc.tensor.matmul(
                        qp,
                        _r(Wq_sb[:, kt, mt * 128 : (mt + 1) * 128]),
                        _r(k_sb[:, kt, :]),
                        start=(kt == 0),
                        stop=(kt == 3),
                    )
                nc.scalar.copy(_r(kq_sb[:, mt, :]), qp)

            # ---------------- per-batch attention + output ----------------
            for b in range(BPC):
                sl = slice(b * WA + 1, (b + 1) * WA)  # this batch's 25 cols

                # vT_aug [26, 1024]: row 0 = gamma*bv, rows 1..25 = gamma*v^T
                vT_aug = small.tile([WA, Cr], FP, tag="vT")
                for nof, nn in [(0, 512), (512, 512)]:
                    vp = ps1.tile([WA, 512], FP, tag="ps1")
                    nc.tensor.matmul(
                        vp[:, 0:nn],
                        _r(onehot[0:1, :]),
                        _r(gbv_row[0:1, nof : nof + nn]),
                        start=True,
                        stop=False,
                    )
                    for ct in range(2):
                        nc.tensor.matmul(
                            vp[:, 0:nn],
                            _r(pooled[:, ct, b * WA : (b + 1) * WA]),
                            _r(WvTg_sb[:, ct, nof : nof + nn]),
                            start=False,
                            stop=(ct == 1),
                        )
                    nc.scalar.copy(_r(vT_aug[:, nof : nof + nn]), vp[:, 0:nn])

                # eT [25, 784] = kq^T-contraction of x + be ⊗ 1
                eT = psA.tile([Ws, HW], FP, tag="psA")
                for nof, nn in _nt_slices():
                    for kt in range(8):
                        nc.tensor.matmul(
                            eT[:, nof : nof + nn],
                            _r(kq_sb[:, kt, sl]),
                            _r(x_sbs[b][kt // 4][:, kt % 4, nof : nof + nn]),
                            start=(kt == 0),
                            stop=False,
                        )
                    nc.tensor.matmul(
                        eT[:, nof : nof + nn],
                        _r(beT[0:1, sl]),
                        _r(ones_row[0:1, 0:nn]),
                        start=False,
                        stop=True,
                    )

                # E = exp(eT)  (no max-subtract: |energy| <~ 30, fp32-safe)
                E_sb = small.tile([Ws, HW], FP, tag="E")
                nc.scalar.activation(_r(E_sb), eT, func=AF.Exp, bias=0.0, scale=1.0)

                # Eaug = [1|I]^T E : row 0 = colsums, rows 1..25 = E
                Eaug = psA.tile([WA, HW], FP, tag="psA")
                for nof, nn in _nt_slices():
                    nc.tensor.matmul(
                        Eaug[:, nof : nof + nn],
                        _r(aug_sel),
                        _r(E_sb[:, nof : nof + nn]),
                        start=True,
                        stop=True,
                    )
                Eaug_sb = small.tile([WA, HW], FP, tag="Eaug")
                nc.scalar.copy(_r(Eaug_sb), Eaug)

                # S = ones ⊗ colsums (PE broadcast), then wide approx
                # reciprocal (fp32-only op), then attT = Eaug * (1/S):
                # row 0 = s/s = 1, rows 1..25 = att^T
                Sb = psA.tile([WA, HW], FP, tag="psA")
                for nof, nn in _nt_slices():
                    nc.tensor.matmul(
                        Sb[:, nof : nof + nn],
                        _r(ones_row[0:1, 0:WA]),
                        _r(Eaug_sb[0:1, nof : nof + nn]),
                        start=True,
                        stop=True,
                    )
                Rb = small.tile([WA, HW], FP, tag="Rb")
                nc.vector.reciprocal_approx_fast(Rb, Sb)
                attT = small.tile([WA, HW], FP, tag="attT")
                nc.vector.tensor_mul(_r(attT), Eaug_sb, Rb)

                # out = vT_aug.T @ attT_aug + x_rgb
                for rt in range(8):
                    op = psop.tile([128, HW], FP, tag="op")
                    for nof, nn in _nt_slices():
                        nc.tensor.matmul(
                            op[:, nof : nof + nn],
                            _r(vT_aug[:, rt * 128 : (rt + 1) * 128]),
                            _r(attT[:, nof : nof + nn]),
                            start=True,
                            stop=True,
                        )
                    o_sb = outp.tile([128, HW], FP, tag="o")
                    nc.vector.tensor_add(
                        o_sb, op, x_sbs[b][rt // 4][:, rt % 4, :]
                    )
                    nc.sync.dma_start(
                        out_d[b].rearrange("(t p) n -> p t n", p=128)[:, rt, :], o_sb
                    )

    nc.compile()
    return nc


_NC = None


def _get_nc():
    global _NC
    if _NC is None:
        _NC = _build()
    return _NC


def kernel(x_rgb, x_skel, Wq, bq, Wk, bk, Wv, bv, gamma):
    nc = _get_nc()
    xr = np.ascontiguousarray(x_rgb, dtype=np.float32).reshape(B, Cr, HW)
    xs = np.ascontiguousarray(x_skel, dtype=np.float32).reshape(B, Cs, SK)
    shared = {
        "Wq": np.ascontiguousarray(Wq, dtype=np.float32),
        "bq": np.ascontiguousarray(bq, dtype=np.float32),
        "Wk": np.ascontiguousarray(Wk, dtype=np.float32),
        "bk": np.ascontiguousarray(bk, dtype=np.float32),
        "Wv": np.ascontiguousarray(Wv, dtype=np.float32),
        "bv": np.ascontiguousarray(bv, dtype=np.float32),
        "gamma": np.ascontiguousarray(gamma, dtype=np.float32),
    }
    in_maps = [
        {
            "x_rgb": np.ascontiguousarray(xr[c * BPC : (c + 1) * BPC]),
            "x_skel": np.ascontiguousarray(xs[c * BPC : (c + 1) * BPC]),
            **shared,
        }
        for c in range(N_CORES)
    ]
    res = run_bass_kernel_spmd(nc, in_maps, core_ids=list(range(N_CORES)))
    out = np.concatenate([r["out"] for r in res.results], axis=0)
    return out.reshape(B, Cr, H, W).astype(np.float32)


# revision 33
# speedup vs baseline: 2.4835x; 1.0174x over previous
"""CrossModalAttention fused Bass/Tile kernel for Trainium2 (8 NeuronCores).

Math (per batch b):
    pooled = mean_w x_skel[b]                      # [Cs, Ws]
    k  = Wk @ pooled + bk                          # [Ci, Ws]
    q  = Wq @ x_rgb[b] + bq                        # (never materialized)
    energy = q^T k = x_rgb^T (Wq^T k) + 1 (bq^T k) # [HW, Ws]  <- low-rank trick
    att = softmax(energy, axis=-1)
    v  = Wv @ pooled + bv
    out = gamma * (v @ att^T) + x_rgb

Implementation notes:
  * energy is computed transposed (eT = kq^T-contraction, [Ws, HW]) so the PE
    stationary loads are tiny (25 cols) and the streams are wide (512 cols).
  * softmax runs over the partition axis without max-subtraction (energies
    are O(25), exp stays far below fp32 max):
       E = exp(eT); Eaug = [1|I]^T E  (row 0 = colsums, rows 1..25 = E)
       r = 1/Eaug[0]; attT_aug = Eaug * (ones ⊗ r)
    so attT_aug row 0 == 1, which is exactly the weight the folded bv row
    needs in the output matmul.
  * gamma is folded into Wv^T and bv at setup, so the epilogue is a single
    vector add of the residual per output tile.
  * both batches of a core share the k / bq^T k / Wq^T k projections via a
    52-wide concatenated free axis (26 per batch: col 0 is an aug/pad slot).

Sharding: pure data-parallel over batch B=16 -> 2 batches per NeuronCore.
"""

import os
import sys

for _p in ("/opt/trn_rl_repo", "/root/.axon_site/_ro/trn_rl_repo"):
    if os.path.isdir(_p) and _p not in sys.path:
        sys.path.insert(0, _p)

import numpy as np

import concourse.bass as bass  # noqa: F401
import concourse.mybir as mybir
import concourse.tile as tile
from concourse import bacc
from concourse.bass_utils import run_bass_kernel_spmd
from concourse.masks import make_identity

B, Cr, H, W = 16, 1024, 28, 28
Cs, Hs, Ws = 256, 25, 25
Ci = 512
HW = H * W  # 784
SK = Hs * Ws  # 625
N_CORES = 8
BPC = B // N_CORES  # batches per core = 2
WA = Ws + 1  # 26: per-batch block (col/row 0 = aug slot)
NT = (512, 272)  # free-dim tiling of HW=784, bank-aligned
FP = mybir.dt.float32
FPR = mybir.dt.float32r
AX = mybir.AxisListType
AF = mybir.ActivationFunctionType


def _r(ap):
    """View an fp32 AP as float32r: single-pass PE matmul (1 cycle/row for
    moving dims >= 256, vs 4 for plain fp32)."""
    return ap.bitcast(FPR)


def _nt_slices():
    off = 0
    for n in NT:
        yield off, n
        off += n


def _build():
    nc = bacc.Bacc(None, target_bir_lowering=False)

    x_rgb = nc.dram_tensor("x_rgb", [BPC, Cr, HW], FP, kind="ExternalInput")
    x_skel = nc.dram_tensor("x_skel", [BPC, Cs, SK], FP, kind="ExternalInput")
    Wq_d = nc.dram_tensor("Wq", [Ci, Cr], FP, kind="ExternalInput")
    bq_d = nc.dram_tensor("bq", [Ci], FP, kind="ExternalInput")
    Wk_d = nc.dram_tensor("Wk", [Ci, Cs], FP, kind="ExternalInput")
    bk_d = nc.dram_tensor("bk", [Ci], FP, kind="ExternalInput")
    Wv_d = nc.dram_tensor("Wv", [Cr, Cs], FP, kind="ExternalInput")
    bv_d = nc.dram_tensor("bv", [Cr], FP, kind="ExternalInput")
    gamma_d = nc.dram_tensor("gamma", [1], FP, kind="ExternalInput")
    out_d = nc.dram_tensor("out", [BPC, Cr, HW], FP, kind="ExternalOutput")

    with tile.TileContext(nc) as tc:
        with (
            # float32r outputs are 4-byte fp32 storage; only the matmul
            # ingest rounds (TF32-style), so this is not a real low-precision
            # accumulation.
            nc.allow_low_precision(reason="float32r tagging of fp32 tiles"),
            tc.tile_pool(name="const", bufs=1) as const,
            tc.tile_pool(name="wt", bufs=1) as wt,
            tc.tile_pool(name="xp", bufs=2) as xp,
            tc.tile_pool(name="small", bufs=2) as small,
            tc.tile_pool(name="outp", bufs=3) as outp,
            tc.tile_pool(name="psS", bufs=4, space="PSUM") as psS,
            tc.tile_pool(name="psop", bufs=2, space="PSUM") as psop,
        ):
            # ------- input DMAs: weights/consts first (they gate all PE
            # work), then x_skel (gates pooled->k->kq), then the big x_rgb
            # tensors which are only needed as the energy matmuls stream.
            bk_row = const.tile([1, Ci], FP)
            nc.sync.dma_start(_r(bk_row[:]), _r(bk_d[:].rearrange("(o i) -> o i", o=1)))
            bq_col = const.tile([128, 4], FP)
            nc.sync.dma_start(_r(bq_col[:]), _r(bq_d[:].rearrange("(t p) -> p t", p=128)))
            bv_row = const.tile([1, Cr], FP)
            nc.sync.dma_start(bv_row, bv_d[:].rearrange("(o r) -> o r", o=1))
            gamma_b = const.tile([128, 1], FP)
            nc.sync.dma_start(gamma_b, gamma_d[:].to_broadcast([128, 1]))

            Wk_sb = wt.tile([128, 4, Cs], FP)
            nc.sync.dma_start(Wk_sb, Wk_d.rearrange("(t p) c -> p t c", p=128))
            Wv_sb = wt.tile([128, 8, Cs], FP)
            nc.sync.dma_start(Wv_sb, Wv_d.rearrange("(t p) c -> p t c", p=128))
            Wq_sb = wt.tile([128, 4, Cr], FP)
            nc.sync.dma_start(_r(Wq_sb[:]), _r(Wq_d.rearrange("(t p) r -> p t r", p=128)))

            x_sbs, xs_sbs = [], []
            for b in range(BPC):
                xs_sb = xp.tile([128, 2, SK], FP, tag="xs")
                nc.sync.dma_start(xs_sb, x_skel[b].rearrange("(t p) j -> p t j", p=128))
                xs_sbs.append(xs_sb)
            for b in range(BPC):
                halves = []
                for h in range(2):
                    x_sb = xp.tile([128, 4, HW], FP, tag=f"x{h}")
                    nc.sync.dma_start(
                        _r(x_sb[:]),
                        _r(
                            x_rgb[b].rearrange("(t p) n -> p t n", p=128)[
                                :, h * 4 : (h + 1) * 4, :
                            ]
                        ),
                    )
                    halves.append(x_sb)
                x_sbs.append(halves)

            identity = const.tile([128, 128], FP)
            make_identity(nc, identity)
            # memset cannot write float32r; build constants in fp32 scratch
            # and ACT-copy into the f32r-tagged tiles.
            ones_tmp = const.tile([1, 512], FP, tag="ones_tmp")
            nc.vector.memset(ones_tmp, 1.0)
            ones_row = const.tile([1, 512], FP)
            nc.scalar.copy(_r(ones_row), ones_tmp)
            # one-hot row: col 0 = 1 (selects the bias row of vT_aug)
            oh_tmp = const.tile([1, WA], FP, tag="oh_tmp")
            nc.vector.memset(oh_tmp, 0.0)
            nc.vector.memset(oh_tmp[0:1, 0:1], 1.0)
            onehot = const.tile([1, WA], FP)
            nc.scalar.copy(_r(onehot), oh_tmp)
            # aug_sel [25, 26]: col 0 = ones (colsum), cols 1..25 = I_25
            as_tmp = const.tile([Ws, WA], FP, tag="as_tmp")
            nc.gpsimd.memset(as_tmp, 0.0)
            nc.gpsimd.affine_select(
                out=as_tmp[:, 1:WA],
                in_=as_tmp[:, 1:WA],
                compare_op=mybir.AluOpType.not_equal,
                fill=1.0,
                base=0,
                pattern=[[-1, Ws]],
                channel_multiplier=1,
            )
            nc.vector.memset(as_tmp[:, 0:1], 1.0)
            aug_sel = const.tile([Ws, WA], FP)
            nc.scalar.copy(_r(aug_sel), as_tmp)
            zcol2 = const.tile([128, 2], FP)
            nc.vector.memset(zcol2, 0.0)

            # gamma-scaled bits
            g25 = const.tile([128, 1], FP)
            nc.vector.tensor_scalar_mul(g25, gamma_b, 1.0 / Ws)
            gbv_row = const.tile([1, Cr], FP)
            nc.vector.tensor_scalar_mul(_r(gbv_row), bv_row, gamma_b[0:1, 0:1])

            # -------- one-time weight transposes (PE) --------
            # WkT[c, i] = Wk[i, c] / 25 ; WvTg[c, r] = gamma * Wv[r, c] / 25
            WkT_sb = wt.tile([128, 2, Ci], FP)
            for kt in range(4):
                for ct in range(2):
                    tp_ = psS.tile([128, 128], FP, tag="psS")
                    nc.tensor.transpose(
                        tp_, Wk_sb[:, kt, ct * 128 : (ct + 1) * 128], identity
                    )
                    nc.scalar.mul(
                        _r(WkT_sb[:, ct, kt * 128 : (kt + 1) * 128]), tp_, 1.0 / Ws
                    )
            WvTg_sb = wt.tile([128, 2, Cr], FP)
            for rt in range(8):
                for ct in range(2):
                    tp_ = psS.tile([128, 128], FP, tag="psS")
                    nc.tensor.transpose(
                        tp_, Wv_sb[:, rt, ct * 128 : (ct + 1) * 128], identity
                    )
                    nc.scalar.mul(
                        _r(WvTg_sb[:, ct, rt * 128 : (rt + 1) * 128]), tp_, g25[:, 0:1]
                    )

            # ---------------- batch-shared projections ----------------
            # pooled_aug [128, 2, 52]: per-batch 26-block, col 0 zero (aug)
            pooled = small.tile([128, 2, 2 * WA], FP, tag="pooled")
            for b in range(BPC):
                nc.scalar.copy(
                    _r(pooled[:, :, b * WA : b * WA + 1]),
                    zcol2.rearrange("p (t o) -> p t o", o=1),
                )
                nc.vector.reduce_sum(
                    _r(pooled[:, :, b * WA + 1 : (b + 1) * WA]),
                    xs_sbs[b].rearrange("p t (h w) -> p t h w", w=Ws),
                    axis=AX.X,
                )

            # k = WkT.T @ pooled + bk  (both batches) -> [128, 4, 52]
            k_sb = small.tile([128, 4, 2 * WA], FP, tag="k")
            for mt in range(4):
                kp = psS.tile([128, 2 * WA], FP, tag="psS")
                for ct in range(2):
                    nc.tensor.matmul(
                        kp,
                        _r(WkT_sb[:, ct, mt * 128 : (mt + 1) * 128]),
                        _r(pooled[:, ct, :]),
                        start=(ct == 0),
                        stop=False,
                    )
                nc.tensor.matmul(
                    kp,
                    _r(bk_row[0:1, mt * 128 : (mt + 1) * 128]),
                    _r(ones_row[0:1, 0 : 2 * WA]),
                    start=False,
                    stop=True,
                )
                nc.scalar.copy(_r(k_sb[:, mt, :]), kp)

            # beT[0, .] = bq^T k  (both batches)
            beT = small.tile([1, 2 * WA], FP, tag="beT")
            bp = psS.tile([1, 2 * WA], FP, tag="psS")
            for kt in range(4):
                nc.tensor.matmul(
                    bp,
                    _r(bq_col[:, kt : kt + 1]),
                    _r(k_sb[:, kt, :]),
                    start=(kt == 0),
                    stop=(kt == 3),
                )
            nc.scalar.copy(_r(beT), bp)

            # kq = Wq.T @ k  (both batches) -> [128, 8, 52], computed as
            # kq^T = k^T-contraction with wide (512) streams at 1 cycle/row,
            # then transposed back 128 cols at a time on the PE.
            kqT_sb = small.tile([2 * WA, Cr], FP, tag="kqT")
            for nof2 in range(2):
                qp = psS.tile([2 * WA, 512], FP, tag="psS")
                for kt in range(4):
                    nc.tensor.matmul(
                        qp,
                        _r(k_sb[:, kt, :]),
                        _r(Wq_sb[:, kt, nof2 * 512 : (nof2 + 1) * 512]),
                        start=(kt == 0),
                        stop=(kt == 3),
                    )
                nc.scalar.copy(_r(kqT_sb[:, nof2 * 512 : (nof2 + 1) * 512]), qp)
            kq_sb = small.tile([128, 8, 2 * WA], FP, tag="kq")
            for mt in range(8):
                qp2 = psS.tile([128, 2 * WA], FP, tag="psS")
                nc.tensor.transpose(
                    qp2[:, 0 : 2 * WA],
                    kqT_sb[:, mt * 128 : (mt + 1) * 128],
                    identity[0 : 2 * WA, 0 : 2 * WA],
                )
                nc.scalar.copy(_r(kq_sb[:, mt, :]), qp2)

            # ------- vT for both batches (only needs pooled + weights) -------
            vT_augs = []
            for b in range(BPC):
                # vT_aug [26, 1024]: row 0 = gamma*bv, rows 1..25 = gamma*v^T
                vT_aug = small.tile([WA, Cr], FP, tag="vT")
                for nof, nn in [(0, 512), (512, 512)]:
                    vp = psS.tile([WA, 512], FP, tag="psS")
                    nc.tensor.matmul(
                        vp[:, 0:nn],
                        _r(onehot[0:1, :]),
                        _r(gbv_row[0:1, nof : nof + nn]),
                        start=True,
                        stop=False,
                    )
                    for ct in range(2):
                        nc.tensor.matmul(
                            vp[:, 0:nn],
                            _r(pooled[:, ct, b * WA : (b + 1) * WA]),
                            _r(WvTg_sb[:, ct, nof : nof + nn]),
                            start=False,
                            stop=(ct == 1),
                        )
                    nc.scalar.copy(_r(vT_aug[:, nof : nof + nn]), vp[:, 0:nn])
                vT_augs.append(vT_aug)

            # ------- attention for both batches, then both output phases.
            # Emitting b1's attention before b0's output lets its energy
            # matmuls fill PE gaps while b0's softmax runs on ACT/DVE.
            attTs = []
            for b in range(BPC):
                sl = slice(b * WA + 1, (b + 1) * WA)  # this batch's 25 cols
                E_sb = small.tile([Ws, HW], FP, tag="E")
                Eaug_sb = small.tile([WA, HW], FP, tag="Eaug")
                Rb = small.tile([WA, HW], FP, tag="Rb")
                attT = small.tile([WA, HW], FP, tag="attT")
                for nof, nn in _nt_slices():
                    eT = psS.tile([Ws, 512], FP, tag="psS")
                    for kt in range(8):
                        nc.tensor.matmul(
                            eT[:, 0:nn],
                            _r(kq_sb[:, kt, sl]),
                            _r(x_sbs[b][kt // 4][:, kt % 4, nof : nof + nn]),
                            start=(kt == 0),
                            stop=False,
                        )
                    nc.tensor.matmul(
                        eT[:, 0:nn],
                        _r(beT[0:1, sl]),
                        _r(ones_row[0:1, 0:nn]),
                        start=False,
                        stop=True,
                    )
                    # E = exp(eT)  (no max-subtract: |energy| <~ 30, fp32-safe)
                    nc.scalar.activation(
                        _r(E_sb[:, nof : nof + nn]),
                        eT[:, 0:nn],
                        func=AF.Exp,
                        bias=0.0,
                        scale=1.0,
                    )
                    # Eaug = [1|I]^T E : row 0 = colsums, rows 1..25 = E
                    Eaug = psS.tile([WA, 512], FP, tag="psS")
                    nc.tensor.matmul(
                        Eaug[:, 0:nn],
                        _r(aug_sel),
                        _r(E_sb[:, nof : nof + nn]),
                        start=True,
                        stop=True,
                    )
                    nc.scalar.copy(_r(Eaug_sb[:, nof : nof + nn]), Eaug[:, 0:nn])
                    # S = ones ⊗ colsums (PE broadcast), wide approx reciprocal
                    # (fp32-only op), then attT = Eaug * (1/S):
                    # row 0 = s/s = 1, rows 1..25 = att^T
                    Sb = psS.tile([WA, 512], FP, tag="psS")
                    nc.tensor.matmul(
                        Sb[:, 0:nn],
                        _r(ones_row[0:1, 0:WA]),
                        _r(Eaug_sb[0:1, nof : nof + nn]),
                        start=True,
                        stop=True,
                    )
                    nc.vector.reciprocal_approx_fast(
                        Rb[:, nof : nof + nn], Sb[:, 0:nn]
                    )
                    nc.vector.tensor_mul(
                        _r(attT[:, nof : nof + nn]),
                        Eaug_sb[:, nof : nof + nn],
                        Rb[:, nof : nof + nn],
                    )
                attTs.append(attT)

            for b in range(BPC):
                # out = vT_aug.T @ attT_aug + x_rgb
                for rt in range(8):
                    op = psop.tile([128, HW], FP, tag="op")
                    for nof, nn in _nt_slices():
                        nc.tensor.matmul(
                            op[:, nof : nof + nn],
                            _r(vT_augs[b][:, rt * 128 : (rt + 1) * 128]),
                            _r(attTs[b][:, nof : nof + nn]),
                            start=True,
                            stop=True,
                        )
                    o_sb = outp.tile([128, HW], FP, tag="o")
                    nc.vector.tensor_add(
                        o_sb, op, x_sbs[b][rt // 4][:, rt % 4, :]
                    )
                    nc.sync.dma_start(
                        out_d[b].rearrange("(t p) n -> p t n", p=128)[:, rt, :], o_sb
                    )

    nc.compile()
    return nc


_NC = None


def _get_nc():
    global _NC
    if _NC is None:
        _NC = _build()
    return _NC


def kernel(x_rgb, x_skel, Wq, bq, Wk, bk, Wv, bv, gamma):
    nc = _get_nc()
    xr = np.ascontiguousarray(x_rgb, dtype=np.float32).reshape(B, Cr, HW)
    xs = np.ascontiguousarray(x_skel, dtype=np.float32).reshape(B, Cs, SK)
    shared = {
        "Wq": np.ascontiguousarray(Wq, dtype=np.float32),
        "bq": np.ascontiguousarray(bq, dtype=np.float32),
        "Wk": np.ascontiguousarray(Wk, dtype=np.float32),
        "bk": np.ascontiguousarray(bk, dtype=np.float32),
        "Wv": np.ascontiguousarray(Wv, dtype=np.float32),
        "bv": np.ascontiguousarray(bv, dtype=np.float32),
        "gamma": np.ascontiguousarray(gamma, dtype=np.float32),
    }
    in_maps = [
        {
            "x_rgb": np.ascontiguousarray(xr[c * BPC : (c + 1) * BPC]),
            "x_skel": np.ascontiguousarray(xs[c * BPC : (c + 1) * BPC]),
            **shared,
        }
        for c in range(N_CORES)
    ]
    res = run_bass_kernel_spmd(nc, in_maps, core_ids=list(range(N_CORES)))
    out = np.concatenate([r["out"] for r in res.results], axis=0)
    return out.reshape(B, Cr, H, W).astype(np.float32)


# revision 34
# speedup vs baseline: 2.5517x; 1.0275x over previous
"""CrossModalAttention fused Bass/Tile kernel for Trainium2 (8 NeuronCores).

Math (per batch b):
    pooled = mean_w x_skel[b]                      # [Cs, Ws]
    k  = Wk @ pooled + bk                          # [Ci, Ws]
    q  = Wq @ x_rgb[b] + bq                        # (never materialized)
    energy = q^T k = x_rgb^T (Wq^T k) + 1 (bq^T k) # [HW, Ws]  <- low-rank trick
    att = softmax(energy, axis=-1)
    v  = Wv @ pooled + bv
    out = gamma * (v @ att^T) + x_rgb

Implementation notes:
  * energy is computed transposed (eT = kq^T-contraction, [Ws, HW]) so the PE
    stationary loads are tiny (25 cols) and the streams are wide (512 cols).
  * softmax runs over the partition axis without max-subtraction (energies
    are O(25), exp stays far below fp32 max):
       E = exp(eT); Eaug = [1|I]^T E  (row 0 = colsums, rows 1..25 = E)
       r = 1/Eaug[0]; attT_aug = Eaug * (ones ⊗ r)
    so attT_aug row 0 == 1, which is exactly the weight the folded bv row
    needs in the output matmul.
  * gamma is folded into Wv^T and bv at setup, so the epilogue is a single
    vector add of the residual per output tile.
  * both batches of a core share the k / bq^T k / Wq^T k projections via a
    52-wide concatenated free axis (26 per batch: col 0 is an aug/pad slot).

Sharding: pure data-parallel over batch B=16 -> 2 batches per NeuronCore.
"""

import os
import sys

for _p in ("/opt/trn_rl_repo", "/root/.axon_site/_ro/trn_rl_repo"):
    if os.path.isdir(_p) and _p not in sys.path:
        sys.path.insert(0, _p)

import numpy as np

import concourse.bass as bass  # noqa: F401
import concourse.mybir as mybir
import concourse.tile as tile
from concourse import bacc
from concourse.bass_utils import run_bass_kernel_spmd
from concourse.masks import make_identity

B, Cr, H, W = 16, 1024, 28, 28
Cs, Hs, Ws = 256, 25, 25
Ci = 512
HW = H * W  # 784
SK = Hs * Ws  # 625
N_CORES = 8
BPC = B // N_CORES  # batches per core = 2
WA = Ws + 1  # 26: per-batch block (col/row 0 = aug slot)
NT = (512, 272)  # free-dim tiling of HW=784, bank-aligned
FP = mybir.dt.float32
FPR = mybir.dt.float32r
AX = mybir.AxisListType
AF = mybir.ActivationFunctionType


def _r(ap):
    """View an fp32 AP as float32r: single-pass PE matmul (1 cycle/row for
    moving dims >= 256, vs 4 for plain fp32)."""
    return ap.bitcast(FPR)


def _nt_slices():
    off = 0
    for n in NT:
        yield off, n
        off += n


def _build():
    nc = bacc.Bacc(None, target_bir_lowering=False)

    x_rgb = nc.dram_tensor("x_rgb", [BPC, Cr, HW], FP, kind="ExternalInput")
    x_skel = nc.dram_tensor("x_skel", [BPC, Cs, SK], FP, kind="ExternalInput")
    Wq_d = nc.dram_tensor("Wq", [Ci, Cr], FP, kind="ExternalInput")
    bq_d = nc.dram_tensor("bq", [Ci], FP, kind="ExternalInput")
    Wk_d = nc.dram_tensor("Wk", [Ci, Cs], FP, kind="ExternalInput")
    bk_d = nc.dram_tensor("bk", [Ci], FP, kind="ExternalInput")
    Wv_d = nc.dram_tensor("Wv", [Cr, Cs], FP, kind="ExternalInput")
    bv_d = nc.dram_tensor("bv", [Cr], FP, kind="ExternalInput")
    gamma_d = nc.dram_tensor("gamma", [1], FP, kind="ExternalInput")
    out_d = nc.dram_tensor("out", [BPC, Cr, HW], FP, kind="ExternalOutput")

    with tile.TileContext(nc) as tc:
        with (
            # float32r outputs are 4-byte fp32 storage; only the matmul
            # ingest rounds (TF32-style), so this is not a real low-precision
            # accumulation.
            nc.allow_low_precision(reason="float32r tagging of fp32 tiles"),
            tc.tile_pool(name="const", bufs=1) as const,
            tc.tile_pool(name="wt", bufs=1) as wt,
            tc.tile_pool(name="xp", bufs=2) as xp,
            tc.tile_pool(name="small", bufs=2) as small,
            tc.tile_pool(name="outp", bufs=3) as outp,
            tc.tile_pool(name="psS", bufs=4, space="PSUM") as psS,
            tc.tile_pool(name="psop", bufs=2, space="PSUM") as psop,
        ):
            # ------- input DMAs: weights/consts first (they gate all PE
            # work), then x_skel (gates pooled->k->kq), then the big x_rgb
            # tensors which are only needed as the energy matmuls stream.
            bk_row = const.tile([1, Ci], FP)
            nc.sync.dma_start(_r(bk_row[:]), _r(bk_d[:].rearrange("(o i) -> o i", o=1)))
            bq_col = const.tile([128, 4], FP)
            nc.sync.dma_start(_r(bq_col[:]), _r(bq_d[:].rearrange("(t p) -> p t", p=128)))
            bv_row = const.tile([1, Cr], FP)
            nc.sync.dma_start(bv_row, bv_d[:].rearrange("(o r) -> o r", o=1))
            gamma_b = const.tile([128, 1], FP)
            nc.sync.dma_start(gamma_b, gamma_d[:].to_broadcast([128, 1]))

            Wk_sb = wt.tile([128, 4, Cs], FP)
            nc.sync.dma_start(Wk_sb, Wk_d.rearrange("(t p) c -> p t c", p=128))
            x_sbs, xs_sbs = [], []
            for b in range(BPC):
                xs_sb = xp.tile([128, 2, SK], FP, tag="xs")
                nc.sync.dma_start(xs_sb, x_skel[b].rearrange("(t p) j -> p t j", p=128))
                xs_sbs.append(xs_sb)
            Wv_sb = wt.tile([128, 8, Cs], FP)
            nc.sync.dma_start(Wv_sb, Wv_d.rearrange("(t p) c -> p t c", p=128))
            Wq_sb = wt.tile([128, 4, Cr], FP)
            nc.sync.dma_start(_r(Wq_sb[:]), _r(Wq_d.rearrange("(t p) r -> p t r", p=128)))

            for b in range(BPC):
                halves = []
                for h in range(2):
                    x_sb = xp.tile([128, 4, HW], FP, tag=f"x{h}")
                    nc.sync.dma_start(
                        _r(x_sb[:]),
                        _r(
                            x_rgb[b].rearrange("(t p) n -> p t n", p=128)[
                                :, h * 4 : (h + 1) * 4, :
                            ]
                        ),
                    )
                    halves.append(x_sb)
                x_sbs.append(halves)

            identity = const.tile([128, 128], FP)
            make_identity(nc, identity)
            # memset cannot write float32r; build constants in fp32 scratch
            # and ACT-copy into the f32r-tagged tiles.
            ones_tmp = const.tile([1, 512], FP, tag="ones_tmp")
            nc.vector.memset(ones_tmp, 1.0)
            ones_row = const.tile([1, 512], FP)
            nc.scalar.copy(_r(ones_row), ones_tmp)
            # one-hot row: col 0 = 1 (selects the bias row of vT_aug)
            oh_tmp = const.tile([1, WA], FP, tag="oh_tmp")
            nc.vector.memset(oh_tmp, 0.0)
            nc.vector.memset(oh_tmp[0:1, 0:1], 1.0)
            onehot = const.tile([1, WA], FP)
            nc.scalar.copy(_r(onehot), oh_tmp)
            # aug_sel [25, 26]: col 0 = ones (colsum), cols 1..25 = I_25
            as_tmp = const.tile([Ws, WA], FP, tag="as_tmp")
            nc.gpsimd.memset(as_tmp, 0.0)
            nc.gpsimd.affine_select(
                out=as_tmp[:, 1:WA],
                in_=as_tmp[:, 1:WA],
                compare_op=mybir.AluOpType.not_equal,
                fill=1.0,
                base=0,
                pattern=[[-1, Ws]],
                channel_multiplier=1,
            )
            nc.vector.memset(as_tmp[:, 0:1], 1.0)
            aug_sel = const.tile([Ws, WA], FP)
            nc.scalar.copy(_r(aug_sel), as_tmp)
            zcol2 = const.tile([128, 2], FP)
            nc.vector.memset(zcol2, 0.0)

            # gamma-scaled bits
            g25 = const.tile([128, 1], FP)
            nc.vector.tensor_scalar_mul(g25, gamma_b, 1.0 / Ws)
            gbv_row = const.tile([1, Cr], FP)
            nc.vector.tensor_scalar_mul(_r(gbv_row), bv_row, gamma_b[0:1, 0:1])

            # -------- one-time weight transposes (PE) --------
            # WkT[c, i] = Wk[i, c] / 25 ; WvTg[c, r] = gamma * Wv[r, c] / 25
            WkT_sb = wt.tile([128, 2, Ci], FP)
            for kt in range(4):
                for ct in range(2):
                    tp_ = psS.tile([128, 128], FP, tag="psS")
                    nc.tensor.transpose(
                        tp_, Wk_sb[:, kt, ct * 128 : (ct + 1) * 128], identity
                    )
                    nc.scalar.mul(
                        _r(WkT_sb[:, ct, kt * 128 : (kt + 1) * 128]), tp_, 1.0 / Ws
                    )
            WvTg_sb = wt.tile([128, 2, Cr], FP)
            for rt in range(8):
                for ct in range(2):
                    tp_ = psS.tile([128, 128], FP, tag="psS")
                    nc.tensor.transpose(
                        tp_, Wv_sb[:, rt, ct * 128 : (ct + 1) * 128], identity
                    )
                    nc.scalar.mul(
                        _r(WvTg_sb[:, ct, rt * 128 : (rt + 1) * 128]), tp_, g25[:, 0:1]
                    )

            # ---------------- batch-shared projections ----------------
            # pooled_aug [128, 2, 52]: per-batch 26-block, col 0 zero (aug)
            pooled = small.tile([128, 2, 2 * WA], FP, tag="pooled")
            for b in range(BPC):
                nc.scalar.copy(
                    _r(pooled[:, :, b * WA : b * WA + 1]),
                    zcol2.rearrange("p (t o) -> p t o", o=1),
                )
                nc.vector.reduce_sum(
                    _r(pooled[:, :, b * WA + 1 : (b + 1) * WA]),
                    xs_sbs[b].rearrange("p t (h w) -> p t h w", w=Ws),
                    axis=AX.X,
                )

            # k = WkT.T @ pooled + bk  (both batches) -> [128, 4, 52]
            k_sb = small.tile([128, 4, 2 * WA], FP, tag="k")
            for mt in range(4):
                kp = psS.tile([128, 2 * WA], FP, tag="psS")
                for ct in range(2):
                    nc.tensor.matmul(
                        kp,
                        _r(WkT_sb[:, ct, mt * 128 : (mt + 1) * 128]),
                        _r(pooled[:, ct, :]),
                        start=(ct == 0),
                        stop=False,
                    )
                nc.tensor.matmul(
                    kp,
                    _r(bk_row[0:1, mt * 128 : (mt + 1) * 128]),
                    _r(ones_row[0:1, 0 : 2 * WA]),
                    start=False,
                    stop=True,
                )
                nc.scalar.copy(_r(k_sb[:, mt, :]), kp)

            # beT[0, .] = bq^T k  (both batches)
            beT = small.tile([1, 2 * WA], FP, tag="beT")
            bp = psS.tile([1, 2 * WA], FP, tag="psS")
            for kt in range(4):
                nc.tensor.matmul(
                    bp,
                    _r(bq_col[:, kt : kt + 1]),
                    _r(k_sb[:, kt, :]),
                    start=(kt == 0),
                    stop=(kt == 3),
                )
            nc.scalar.copy(_r(beT), bp)

            # kq = Wq.T @ k  (both batches) -> [128, 8, 52], computed as
            # kq^T = k^T-contraction with wide (512) streams at 1 cycle/row,
            # then transposed back 128 cols at a time on the PE.
            kqT_sb = small.tile([2 * WA, Cr], FP, tag="kqT")
            for nof2 in range(2):
                qp = psS.tile([2 * WA, 512], FP, tag="psS")
                for kt in range(4):
                    nc.tensor.matmul(
                        qp,
                        _r(k_sb[:, kt, :]),
                        _r(Wq_sb[:, kt, nof2 * 512 : (nof2 + 1) * 512]),
                        start=(kt == 0),
                        stop=(kt == 3),
                    )
                nc.scalar.copy(_r(kqT_sb[:, nof2 * 512 : (nof2 + 1) * 512]), qp)
            kq_sb = small.tile([128, 8, 2 * WA], FP, tag="kq")
            for mt in range(8):
                qp2 = psS.tile([128, 2 * WA], FP, tag="psS")
                nc.tensor.transpose(
                    qp2[:, 0 : 2 * WA],
                    kqT_sb[:, mt * 128 : (mt + 1) * 128],
                    identity[0 : 2 * WA, 0 : 2 * WA],
                )
                nc.scalar.copy(_r(kq_sb[:, mt, :]), qp2)

            # ------- vT for both batches (only needs pooled + weights) -------
            vT_augs = []
            for b in range(BPC):
                # vT_aug [26, 1024]: row 0 = gamma*bv, rows 1..25 = gamma*v^T
                vT_aug = small.tile([WA, Cr], FP, tag="vT")
                for nof, nn in [(0, 512), (512, 512)]:
                    vp = psS.tile([WA, 512], FP, tag="psS")
                    nc.tensor.matmul(
                        vp[:, 0:nn],
                        _r(onehot[0:1, :]),
                        _r(gbv_row[0:1, nof : nof + nn]),
                        start=True,
                        stop=False,
                    )
                    for ct in range(2):
                        nc.tensor.matmul(
                            vp[:, 0:nn],
                            _r(pooled[:, ct, b * WA : (b + 1) * WA]),
                            _r(WvTg_sb[:, ct, nof : nof + nn]),
                            start=False,
                            stop=(ct == 1),
                        )
                    nc.scalar.copy(_r(vT_aug[:, nof : nof + nn]), vp[:, 0:nn])
                vT_augs.append(vT_aug)

            # ------- attention for both batches, then both output phases.
            # Emitting b1's attention before b0's output lets its energy
            # matmuls fill PE gaps while b0's softmax runs on ACT/DVE.
            attTs = []
            for b in range(BPC):
                sl = slice(b * WA + 1, (b + 1) * WA)  # this batch's 25 cols
                E_sb = small.tile([Ws, HW], FP, tag="E")
                Eaug_sb = small.tile([WA, HW], FP, tag="Eaug")
                Rb = small.tile([WA, HW], FP, tag="Rb")
                attT = small.tile([WA, HW], FP, tag="attT")
                for nof, nn in _nt_slices():
                    eT = psS.tile([Ws, 512], FP, tag="psS")
                    for kt in range(8):
                        nc.tensor.matmul(
                            eT[:, 0:nn],
                            _r(kq_sb[:, kt, sl]),
                            _r(x_sbs[b][kt // 4][:, kt % 4, nof : nof + nn]),
                            start=(kt == 0),
                            stop=False,
                        )
                    nc.tensor.matmul(
                        eT[:, 0:nn],
                        _r(beT[0:1, sl]),
                        _r(ones_row[0:1, 0:nn]),
                        start=False,
                        stop=True,
                    )
                    # E = exp(eT)  (no max-subtract: |energy| <~ 30, fp32-safe)
                    nc.scalar.activation(
                        _r(E_sb[:, nof : nof + nn]),
                        eT[:, 0:nn],
                        func=AF.Exp,
                        bias=0.0,
                        scale=1.0,
                    )
                    # Eaug = [1|I]^T E : row 0 = colsums, rows 1..25 = E
                    Eaug = psS.tile([WA, 512], FP, tag="psS")
                    nc.tensor.matmul(
                        Eaug[:, 0:nn],
                        _r(aug_sel),
                        _r(E_sb[:, nof : nof + nn]),
                        start=True,
                        stop=True,
                    )
                    nc.scalar.copy(_r(Eaug_sb[:, nof : nof + nn]), Eaug[:, 0:nn])
                    # S = ones ⊗ colsums (PE broadcast), wide approx reciprocal
                    # (fp32-only op), then attT = Eaug * (1/S):
                    # row 0 = s/s = 1, rows 1..25 = att^T
                    Sb = psS.tile([WA, 512], FP, tag="psS")
                    nc.tensor.matmul(
                        Sb[:, 0:nn],
                        _r(ones_row[0:1, 0:WA]),
                        _r(Eaug_sb[0:1, nof : nof + nn]),
                        start=True,
                        stop=True,
                    )
                    nc.vector.reciprocal_approx_fast(
                        Rb[:, nof : nof + nn], Sb[:, 0:nn]
                    )
                    nc.vector.tensor_mul(
                        _r(attT[:, nof : nof + nn]),
                        Eaug_sb[:, nof : nof + nn],
                        Rb[:, nof : nof + nn],
                    )
                attTs.append(attT)

            for b in range(BPC):
                # out = vT_aug.T @ attT_aug + x_rgb
                for rt in range(8):
                    op = psop.tile([128, HW], FP, tag="op")
                    for nof, nn in _nt_slices():
                        nc.tensor.matmul(
                            op[:, nof : nof + nn],
                            _r(vT_augs[b][:, rt * 128 : (rt + 1) * 128]),
                            _r(attTs[b][:, nof : nof + nn]),
                            start=True,
                            stop=True,
                        )
                    o_sb = outp.tile([128, HW], FP, tag="o")
                    nc.vector.tensor_add(
                        o_sb, op, x_sbs[b][rt // 4][:, rt % 4, :]
                    )
                    nc.sync.dma_start(
                        out_d[b].rearrange("(t p) n -> p t n", p=128)[:, rt, :], o_sb
                    )

    nc.compile()
    return nc


_NC = None


def _get_nc():
    global _NC
    if _NC is None:
        _NC = _build()
    return _NC


def kernel(x_rgb, x_skel, Wq, bq, Wk, bk, Wv, bv, gamma):
    nc = _get_nc()
    xr = np.ascontiguousarray(x_rgb, dtype=np.float32).reshape(B, Cr, HW)
    xs = np.ascontiguousarray(x_skel, dtype=np.float32).reshape(B, Cs, SK)
    shared = {
        "Wq": np.ascontiguousarray(Wq, dtype=np.float32),
        "bq": np.ascontiguousarray(bq, dtype=np.float32),
        "Wk": np.ascontiguousarray(Wk, dtype=np.float32),
        "bk": np.ascontiguousarray(bk, dtype=np.float32),
        "Wv": np.ascontiguousarray(Wv, dtype=np.float32),
        "bv": np.ascontiguousarray(bv, dtype=np.float32),
        "gamma": np.ascontiguousarray(gamma, dtype=np.float32),
    }
    in_maps = [
        {
            "x_rgb": np.ascontiguousarray(xr[c * BPC : (c + 1) * BPC]),
            "x_skel": np.ascontiguousarray(xs[c * BPC : (c + 1) * BPC]),
            **shared,
        }
        for c in range(N_CORES)
    ]
    res = run_bass_kernel_spmd(nc, in_maps, core_ids=list(range(N_CORES)))
    out = np.concatenate([r["out"] for r in res.results], axis=0)
    return out.reshape(B, Cr, H, W).astype(np.float32)


# revision 35
# speedup vs baseline: 2.9030x; 1.1377x over previous
"""CrossModalAttention fused Bass/Tile kernel for Trainium2 (8 NeuronCores).

Math (per batch b):
    pooled = mean_w x_skel[b]                      # [Cs, Ws]
    k  = Wk @ pooled + bk                          # [Ci, Ws]
    q  = Wq @ x_rgb[b] + bq                        # (never materialized)
    energy = q^T k = x_rgb^T (Wq^T k) + 1 (bq^T k) # [HW, Ws]  <- low-rank trick
    att = softmax(energy, axis=-1)
    v  = Wv @ pooled + bv
    out = gamma * (v @ att^T) + x_rgb

Weight-only host folds (exact algebra, fp64 accumulation):
    Wkq = (Wk/25)^T Wq   [Cs, Cr]   so  Wq^T k = Wkq^T pooled_sum + bkq
    bkq = Wq^T bk        [Cr]
    u   = Wk^T bq / 25   [Cs]       so  bq^T k = u^T pooled_sum + cbb
    cbb = bq . bk        scalar
    WvT = (Wv/25)^T      [Cs, Cr]   (gamma applied on device)

Device-side structure per core (2 batches):
  * pooled_sum via one vector reduce per batch (1/25 folded into weights).
  * kq^T = pooled^T Wkq + 1 x bkq with wide 512-col f32r streams, then PE
    transposes back to kq [r-tiles, 52] (both batches share everything).
  * energy computed transposed (eT [Ws, HW]): stationary loads are 25 cols,
    streams are 512/272 (f32r, 1 cycle/row).
  * softmax over the partition axis without max-subtraction (energies are
    O(25), exp stays far below fp32 max):
       E = exp(eT); Eaug = [1|I]^T E  (row 0 = colsums, rows 1..25 = E)
       attT_aug = Eaug * (ones x approx_recip(ones x colsums))
    so attT_aug row 0 == 1, exactly the weight the folded bv row needs in
    the output matmul.
  * out = vT_aug^T @ attT_aug (+bv row via one-hot, gamma folded into the
    vT copy) followed by a single vector add of the residual per tile.

Sharding: pure data-parallel over batch B=16 -> 2 batches per NeuronCore.
"""

import os
import sys

for _p in ("/opt/trn_rl_repo", "/root/.axon_site/_ro/trn_rl_repo"):
    if os.path.isdir(_p) and _p not in sys.path:
        sys.path.insert(0, _p)

import numpy as np

import concourse.bass as bass  # noqa: F401
import concourse.mybir as mybir
import concourse.tile as tile
from concourse import bacc
from concourse.bass_utils import run_bass_kernel_spmd
from concourse.masks import make_identity

B, Cr, H, W = 16, 1024, 28, 28
Cs, Hs, Ws = 256, 25, 25
Ci = 512
HW = H * W  # 784
SK = Hs * Ws  # 625
N_CORES = 8
BPC = B // N_CORES  # batches per core = 2
WA = Ws + 1  # 26: per-batch block (col/row 0 = aug slot)
W2 = 2 * WA  # 52
NT = (512, 272)  # free-dim tiling of HW=784, bank-aligned
FP = mybir.dt.float32
FPR = mybir.dt.float32r
AX = mybir.AxisListType
AF = mybir.ActivationFunctionType


def _r(ap):
    """View an fp32 AP as float32r: single-pass PE matmul (1 cycle/row for
    moving dims >= 256, vs 4 for plain fp32)."""
    return ap.bitcast(FPR)


def _nt_slices():
    off = 0
    for n in NT:
        yield off, n
        off += n


def _build():
    nc = bacc.Bacc(None, target_bir_lowering=False)

    x_rgb = nc.dram_tensor("x_rgb", [BPC, Cr, HW], FP, kind="ExternalInput")
    x_skel = nc.dram_tensor("x_skel", [BPC, Cs, SK], FP, kind="ExternalInput")
    Wkq_d = nc.dram_tensor("Wkq", [Cs, Cr], FP, kind="ExternalInput")
    WvT_d = nc.dram_tensor("WvT", [Cs, Cr], FP, kind="ExternalInput")
    bkq_d = nc.dram_tensor("bkq", [Cr], FP, kind="ExternalInput")
    u_d = nc.dram_tensor("u", [Cs], FP, kind="ExternalInput")
    cbb_d = nc.dram_tensor("cbb", [1], FP, kind="ExternalInput")
    bv_d = nc.dram_tensor("bv", [Cr], FP, kind="ExternalInput")
    gamma_d = nc.dram_tensor("gamma", [1], FP, kind="ExternalInput")
    out_d = nc.dram_tensor("out", [BPC, Cr, HW], FP, kind="ExternalOutput")

    with tile.TileContext(nc) as tc:
        with (
            # float32r outputs are 4-byte fp32 storage; only the matmul
            # ingest rounds (TF32-style), so this is not a real low-precision
            # accumulation.
            nc.allow_low_precision(reason="float32r tagging of fp32 tiles"),
            tc.tile_pool(name="const", bufs=1) as const,
            tc.tile_pool(name="wt", bufs=1) as wt,
            tc.tile_pool(name="xp", bufs=2) as xp,
            tc.tile_pool(name="small", bufs=2) as small,
            tc.tile_pool(name="outp", bufs=3) as outp,
            tc.tile_pool(name="psS", bufs=4, space="PSUM") as psS,
            tc.tile_pool(name="psop", bufs=2, space="PSUM") as psop,
        ):
            # ------- input DMAs: consts/weights first (they gate all PE
            # work), then x_skel (gates pooled->kq), then the big x_rgb
            # tensors which are only needed as the energy matmuls stream.
            bkq_row = const.tile([1, Cr], FP)
            nc.sync.dma_start(
                _r(bkq_row[:]), _r(bkq_d[:].rearrange("(o r) -> o r", o=1))
            )
            u_col = const.tile([128, 2], FP)
            nc.sync.dma_start(_r(u_col[:]), _r(u_d[:].rearrange("(t p) -> p t", p=128)))
            cbb_t = const.tile([1, 1], FP)
            nc.sync.dma_start(_r(cbb_t[:]), _r(cbb_d[:].rearrange("(o r) -> o r", o=1)))
            bv_row = const.tile([1, Cr], FP)
            nc.sync.dma_start(_r(bv_row[:]), _r(bv_d[:].rearrange("(o r) -> o r", o=1)))
            gamma_b = const.tile([128, 1], FP)
            nc.sync.dma_start(gamma_b, gamma_d[:].to_broadcast([128, 1]))

            Wkq_sb = wt.tile([128, 2, Cr], FP)
            nc.sync.dma_start(
                _r(Wkq_sb[:]), _r(Wkq_d.rearrange("(t p) r -> p t r", p=128))
            )
            x_sbs, xs_sbs = [], []
            for b in range(BPC):
                xs_sb = xp.tile([128, 2, SK], FP, tag="xs")
                nc.sync.dma_start(xs_sb, x_skel[b].rearrange("(t p) j -> p t j", p=128))
                xs_sbs.append(xs_sb)
            WvT_sb = wt.tile([128, 2, Cr], FP)
            nc.sync.dma_start(
                _r(WvT_sb[:]), _r(WvT_d.rearrange("(t p) r -> p t r", p=128))
            )
            for b in range(BPC):
                halves = []
                for h in range(2):
                    x_sb = xp.tile([128, 4, HW], FP, tag=f"x{h}")
                    nc.sync.dma_start(
                        _r(x_sb[:]),
                        _r(
                            x_rgb[b].rearrange("(t p) n -> p t n", p=128)[
                                :, h * 4 : (h + 1) * 4, :
                            ]
                        ),
                    )
                    halves.append(x_sb)
                x_sbs.append(halves)

            identity = const.tile([128, 128], FP)
            make_identity(nc, identity)
            # memset cannot write float32r; build constants in fp32 scratch
            # and ACT-copy into the f32r-tagged tiles.
            ones_tmp = const.tile([1, 512], FP, tag="ones_tmp")
            nc.vector.memset(ones_tmp, 1.0)
            ones_row = const.tile([1, 512], FP)
            nc.scalar.copy(_r(ones_row), ones_tmp)
            # one-hot row: col 0 = 1 (selects the bias row of vT_aug)
            oh_tmp = const.tile([1, WA], FP, tag="oh_tmp")
            nc.vector.memset(oh_tmp, 0.0)
            nc.vector.memset(oh_tmp[0:1, 0:1], 1.0)
            onehot = const.tile([1, WA], FP)
            nc.scalar.copy(_r(onehot), oh_tmp)
            # aug_sel [25, 26]: col 0 = ones (colsum), cols 1..25 = I_25
            as_tmp = const.tile([Ws, WA], FP, tag="as_tmp")
            nc.gpsimd.memset(as_tmp, 0.0)
            nc.gpsimd.affine_select(
                out=as_tmp[:, 1:WA],
                in_=as_tmp[:, 1:WA],
                compare_op=mybir.AluOpType.not_equal,
                fill=1.0,
                base=0,
                pattern=[[-1, Ws]],
                channel_multiplier=1,
            )
            nc.vector.memset(as_tmp[:, 0:1], 1.0)
            aug_sel = const.tile([Ws, WA], FP)
            nc.scalar.copy(_r(aug_sel), as_tmp)
            zcol2 = const.tile([128, 2], FP)
            nc.vector.memset(zcol2, 0.0)

            # ---------------- batch-shared projections ----------------
            # pooled_aug [128, 2, 52]: per-batch 26-block, col 0 zero (aug)
            pooled = small.tile([128, 2, W2], FP, tag="pooled")
            for b in range(BPC):
                nc.scalar.copy(
                    _r(pooled[:, :, b * WA : b * WA + 1]),
                    zcol2.rearrange("p (t o) -> p t o", o=1),
                )
                nc.vector.reduce_sum(
                    _r(pooled[:, :, b * WA + 1 : (b + 1) * WA]),
                    xs_sbs[b].rearrange("p t (h w) -> p t h w", w=Ws),
                    axis=AX.X,
                )

            # beT[0, .] = u^T pooled + cbb  (= bq^T k, both batches)
            beT = small.tile([1, W2], FP, tag="beT")
            bp = psS.tile([1, W2], FP, tag="psS")
            for ct in range(2):
                nc.tensor.matmul(
                    bp,
                    _r(u_col[:, ct : ct + 1]),
                    _r(pooled[:, ct, :]),
                    start=(ct == 0),
                    stop=False,
                )
            nc.tensor.matmul(
                bp,
                _r(cbb_t),
                _r(ones_row[0:1, 0:W2]),
                start=False,
                stop=True,
            )
            nc.scalar.copy(_r(beT), bp)

            # kq^T = pooled^T Wkq + 1 x bkq  (both batches) -> [52, 1024],
            # wide 512 streams at 1 cycle/row, then PE-transposed back into
            # kq [128, 8, 52] (r on partitions) for the energy matmuls.
            kqT_sb = small.tile([W2, Cr], FP, tag="kqT")
            for nof2 in range(2):
                qp = psS.tile([W2, 512], FP, tag="psS")
                nc.tensor.matmul(
                    qp,
                    _r(ones_row[0:1, 0:W2]),
                    _r(bkq_row[0:1, nof2 * 512 : (nof2 + 1) * 512]),
                    start=True,
                    stop=False,
                )
                for ct in range(2):
                    nc.tensor.matmul(
                        qp,
                        _r(pooled[:, ct, :]),
                        _r(Wkq_sb[:, ct, nof2 * 512 : (nof2 + 1) * 512]),
                        start=False,
                        stop=(ct == 1),
                    )
                nc.scalar.copy(_r(kqT_sb[:, nof2 * 512 : (nof2 + 1) * 512]), qp)
            kq_sb = small.tile([128, 8, W2], FP, tag="kq")
            for mt in range(8):
                qp2 = psS.tile([128, W2], FP, tag="psS")
                nc.tensor.transpose(
                    qp2[:, 0:W2],
                    kqT_sb[:, mt * 128 : (mt + 1) * 128],
                    identity[0:W2, 0:W2],
                )
                nc.scalar.copy(_r(kq_sb[:, mt, :]), qp2)

            # ------- vT for both batches (only needs pooled + weights) -------
            vT_augs = []
            for b in range(BPC):
                # vT_aug [26, 1024]: row 0 = gamma*bv, rows 1..25 = gamma*v^T
                # (gamma applied via the ACT scale on the PSUM->SBUF copy)
                vT_aug = small.tile([WA, Cr], FP, tag="vT")
                for nof, nn in [(0, 512), (512, 512)]:
                    vp = psS.tile([WA, 512], FP, tag="psS")
                    nc.tensor.matmul(
                        vp[:, 0:nn],
                        _r(onehot[0:1, :]),
                        _r(bv_row[0:1, nof : nof + nn]),
                        start=True,
                        stop=False,
                    )
                    for ct in range(2):
                        nc.tensor.matmul(
                            vp[:, 0:nn],
                            _r(pooled[:, ct, b * WA : (b + 1) * WA]),
                            _r(WvT_sb[:, ct, nof : nof + nn]),
                            start=False,
                            stop=(ct == 1),
                        )
                    nc.scalar.mul(
                        _r(vT_aug[:, nof : nof + nn]),
                        vp[:, 0:nn],
                        gamma_b[0:WA, 0:1],
                    )
                vT_augs.append(vT_aug)

            # ------- attention for both batches, then both output phases.
            # Emitting b1's attention before b0's output lets its energy
            # matmuls fill PE gaps while b0's softmax runs on ACT/DVE.
            attTs = []
            for b in range(BPC):
                sl = slice(b * WA + 1, (b + 1) * WA)  # this batch's 25 cols
                E_sb = small.tile([Ws, HW], FP, tag="E")
                Eaug_sb = small.tile([WA, HW], FP, tag="Eaug")
                Rb = small.tile([WA, HW], FP, tag="Rb")
                attT = small.tile([WA, HW], FP, tag="attT")
                for nof, nn in _nt_slices():
                    eT = psS.tile([Ws, 512], FP, tag="psS")
                    for kt in range(8):
                        nc.tensor.matmul(
                            eT[:, 0:nn],
                            _r(kq_sb[:, kt, sl]),
                            _r(x_sbs[b][kt // 4][:, kt % 4, nof : nof + nn]),
                            start=(kt == 0),
                            stop=False,
                        )
                    nc.tensor.matmul(
                        eT[:, 0:nn],
                        _r(beT[0:1, sl]),
                        _r(ones_row[0:1, 0:nn]),
                        start=False,
                        stop=True,
                    )
                    # E = exp(eT)  (no max-subtract: |energy| <~ 30, fp32-safe)
                    nc.scalar.activation(
                        _r(E_sb[:, nof : nof + nn]),
                        eT[:, 0:nn],
                        func=AF.Exp,
                        bias=0.0,
                        scale=1.0,
                    )
                    # Eaug = [1|I]^T E : row 0 = colsums, rows 1..25 = E
                    Eaug = psS.tile([WA, 512], FP, tag="psS")
                    nc.tensor.matmul(
                        Eaug[:, 0:nn],
                        _r(aug_sel),
                        _r(E_sb[:, nof : nof + nn]),
                        start=True,
                        stop=True,
                    )
                    nc.scalar.copy(_r(Eaug_sb[:, nof : nof + nn]), Eaug[:, 0:nn])
                    # S = ones x colsums (PE broadcast), wide approx reciprocal
                    # (fp32-only op), then attT = Eaug * (1/S):
                    # row 0 = s/s = 1, rows 1..25 = att^T
                    Sb = psS.tile([WA, 512], FP, tag="psS")
                    nc.tensor.matmul(
                        Sb[:, 0:nn],
                        _r(ones_row[0:1, 0:WA]),
                        _r(Eaug_sb[0:1, nof : nof + nn]),
                        start=True,
                        stop=True,
                    )
                    nc.vector.reciprocal_approx_fast(
                        Rb[:, nof : nof + nn], Sb[:, 0:nn]
                    )
                    nc.vector.tensor_mul(
                        _r(attT[:, nof : nof + nn]),
                        Eaug_sb[:, nof : nof + nn],
                        Rb[:, nof : nof + nn],
                    )
                attTs.append(attT)

            for b in range(BPC):
                # out = vT_aug.T @ attT_aug + x_rgb
                for rt in range(8):
                    op = psop.tile([128, HW], FP, tag="op")
                    for nof, nn in _nt_slices():
                        nc.tensor.matmul(
                            op[:, nof : nof + nn],
                            _r(vT_augs[b][:, rt * 128 : (rt + 1) * 128]),
                            _r(attTs[b][:, nof : nof + nn]),
                            start=True,
                            stop=True,
                        )
                    o_sb = outp.tile([128, HW], FP, tag="o")
                    nc.vector.tensor_add(
                        o_sb, op, x_sbs[b][rt // 4][:, rt % 4, :]
                    )
                    nc.sync.dma_start(
                        out_d[b].rearrange("(t p) n -> p t n", p=128)[:, rt, :], o_sb
                    )

    nc.compile()
    return nc


_NC = None


def _get_nc():
    global _NC
    if _NC is None:
        _NC = _build()
    return _NC


def prepare_in_maps(x_rgb, x_skel, Wq, bq, Wk, bk, Wv, bv, gamma):
    """Host-side weight folds (weights only, exact algebra in fp64) and
    per-core data-parallel slicing."""
    xr = np.ascontiguousarray(x_rgb, dtype=np.float32).reshape(B, Cr, HW)
    xs = np.ascontiguousarray(x_skel, dtype=np.float32).reshape(B, Cs, SK)
    Wq64 = np.asarray(Wq, dtype=np.float64)
    Wk64 = np.asarray(Wk, dtype=np.float64)
    Wv64 = np.asarray(Wv, dtype=np.float64)
    bq64 = np.asarray(bq, dtype=np.float64)
    bk64 = np.asarray(bk, dtype=np.float64)
    shared = {
        "Wkq": np.ascontiguousarray((Wk64 / Ws).T @ Wq64, dtype=np.float32),
        "WvT": np.ascontiguousarray((Wv64 / Ws).T, dtype=np.float32),
        "bkq": np.ascontiguousarray(Wq64.T @ bk64, dtype=np.float32),
        "u": np.ascontiguousarray(Wk64.T @ bq64 / Ws, dtype=np.float32),
        "cbb": np.array([bq64 @ bk64], dtype=np.float32),
        "bv": np.ascontiguousarray(bv, dtype=np.float32),
        "gamma": np.ascontiguousarray(gamma, dtype=np.float32),
    }
    return [
        {
            "x_rgb": np.ascontiguousarray(xr[c * BPC : (c + 1) * BPC]),
            "x_skel": np.ascontiguousarray(xs[c * BPC : (c + 1) * BPC]),
            **shared,
        }
        for c in range(N_CORES)
    ]


def kernel(x_rgb, x_skel, Wq, bq, Wk, bk, Wv, bv, gamma):
    nc = _get_nc()
    in_maps = prepare_in_maps(x_rgb, x_skel, Wq, bq, Wk, bk, Wv, bv, gamma)
    res = run_bass_kernel_spmd(nc, in_maps, core_ids=list(range(N_CORES)))
    out = np.concatenate([r["out"] for r in res.results], axis=0)
    return out.reshape(B, Cr, H, W).astype(np.float32)
